# revision 14
# baseline (speedup 1.0000x reference)
"""SE(3)-CNN block (TensorProduct -> SE3Conv -> SE3BatchNorm -> BiasRelu) on 8 trn2 cores.

Sharding: core c = (batch b=c//2, out-x-half h=c%2). Each core computes all 64
output channels for 8 of 16 output x-planes of one batch; per-field BN second
moments are combined with a tiny [1,64] AllReduce across all 8 cores.

v2 conv strategy (vs single-plane baseline):
- Every fp32r matmul covers TWO output x-planes = one full psum bank, free
  dim 448-512 (>= 256 keeps fp32r in its fast streaming mode) and half the
  instruction count.
- No M=64 boundary singles: each chunk-1 kx pair block runs its full slot
  range (s=0..9); out-of-range tap contributions land in psum half-banks the
  evacuation never reads (slot 8 L, slot 9) fed from x-padded slab planes.
- chunk2 (channels 128:160, 4 kx taps packed per 128-row contraction) uses
  slots 10-15 (banks 5-7) plus L(0),L(1),L(6),L(7).
- Weight slab dedup: chunk-1 kx blocks stored once in column order
  [0,2 | 4,6 | 1,3 | 5] so every used pair is contiguous (11.2MB vs 24MB).
- Weight DMA on the Activation HWDGE queue, slabs + tail/stat DMAs on SP,
  evacuation adds split across Vector and GpSimd.
"""
import numpy as np
from itertools import product

# problem constants (from spec / reference)
B = 4
S_IN = 16
V_IN = 16
CO = 64          # 16 scalar + 48 vector output channels
CI = 160         # 16 s + 48 v + 96 t_sym
SIZE = 7
PAD = 3
STRIDE = 2
EPS = 1e-5
NCORES = 8
NXS = 23         # x-padded slab planes per core (px 22 feeds garbage slots only)
NZS = 19         # z-padded: zi_slab = zi_global + 2, covering zofs in [-2, 1]
NX2 = 10         # chunk-2 even slab planes (px = 2*xi, xi 0..9)
OXC = 8          # out x-planes per core
PAIRS = [(0, 0), (0, 1), (0, 2), (1, 1), (1, 2), (2, 2)]
VAR_S_DIV = 1.0 / (B * 16 * 16 * 16)
VAR_V_DIV = 1.0 / (B * 3 * 16 * 16 * 16)

SLAB1_SHAPE = (128, NXS, 32, 2, NZS)   # [ci, px, iy, pz, zi]
SLAB2_SHAPE = (128, NX2, 32, 2, NZS)
WA_COLS = 448    # [k0|k2 | k4|k6 | k1|k3 | k5]
W2_COLS = 2 * CO


# ---------------------------------------------------------------- host prep

def _assemble_kernel_sym(inp):
    """Assemble the dense conv kernel [64, 208, 7,7,7] and symmetrize the
    t-block -> [64, 160, 7,7,7]."""
    def blk(w, basis):
        w = np.asarray(w, np.float32)
        basis = np.asarray(basis, np.float32)
        mo, mi, nb = w.shape
        do, di = basis.shape[1], basis.shape[2]
        k = np.einsum('uvb,bijxyz->uivjxyz', w, basis)
        return k.reshape(mo * do, mi * di, SIZE, SIZE, SIZE)

    row_s = np.concatenate([blk(inp['w_ss'], inp['basis_ss']),
                            blk(inp['w_sv'], inp['basis_sv']),
                            blk(inp['w_st'], inp['basis_st'])], axis=1)
    row_v = np.concatenate([blk(inp['w_vs'], inp['basis_vs']),
                            blk(inp['w_vv'], inp['basis_vv']),
                            blk(inp['w_vt'], inp['basis_vt'])], axis=1)
    K = np.concatenate([row_s, row_v], axis=0)  # [64, 208, 7,7,7]

    Ks = np.empty((CO, CI, SIZE, SIZE, SIZE), np.float32)
    Ks[:, :64] = K[:, :64]
    for u in range(16):
        for pi, (i, j) in enumerate(PAIRS):
            src = K[:, 64 + 9 * u + 3 * i + j]
            if i != j:
                src = src + K[:, 64 + 9 * u + 3 * j + i]
            Ks[:, 64 + 6 * u + pi] = src
    return Ks


def _svt_sym(sv):
    """[4,64,32,32,32] -> symmetrized tensor-product features [4,160,32,32,32]."""
    sv = np.asarray(sv, np.float32)
    s = sv[:, :S_IN]
    v = sv[:, S_IN:].reshape(B, V_IN, 3, 32, 32, 32)
    t = np.empty((B, V_IN, 6, 32, 32, 32), np.float32)
    for pi, (i, j) in enumerate(PAIRS):
        t[:, :, pi] = v[:, :, i] * v[:, :, j]
    return np.concatenate([s, v.reshape(B, 48, 32, 32, 32),
                           t.reshape(B, 96, 32, 32, 32)], axis=1)


def _core_slabs(svt, b, h):
    """x/z zero-padded, z-parity-split slabs for core (b, h).

    c1 [128, 23, 32, 2, 19]: plane px holds global ix = px + 16h - 3.
    c2e [128, 10, 32, 2, 19]: block a (rows 32a:32a+32) of plane xi holds
    chunk-2 channels at ix = 2*xi + a + 16h - 3. zi_slab = zi_global + 2.
    """
    sp = svt[b].reshape(CI, 32, 32, 16, 2)   # (ci, x, y, zi, pz); iz = 2*zi + pz
    sp = np.moveaxis(sp, 4, 3)               # (ci, x, y, pz, zi)
    x0 = 16 * h - 3
    c1 = np.zeros(SLAB1_SHAPE, np.float32)
    lo, hi = max(0, x0), min(32, x0 + NXS)
    c1[:, lo - x0:hi - x0, :, :, 2:18] = sp[:128, lo:hi]
    c2e = np.zeros(SLAB2_SHAPE, np.float32)
    for a in range(4):
        for xi in range(NX2):
            ix = 2 * xi + a + x0
            if 0 <= ix < 32:
                c2e[32 * a:32 * a + 32, xi, :, :, 2:18] = sp[128:160, ix]
    return c1, c2e


def _weight_slabs(Ks):
    """(WA, W2). WA [49, 128, 448]: chunk-1 taps in column order
    [k0|k2|k4|k6|k1|k3|k5] (64 cols each); pairs (0,2),(4,6),(1,3) are the
    contiguous 128-col windows at 0, 128, 256; k5 singles at 384.
    W2 [49, 128, 128]: 4-way kx-merged chunk-2 (g=0: kx 0..3 lower 64 cols,
    g=1: kx 4..6 upper)."""
    KXORD = [0, 2, 4, 6, 1, 3, 5]
    WA = np.zeros((49, 128, WA_COLS), np.float32)
    W2 = np.zeros((49, 128, W2_COLS), np.float32)
    for ky, kz in product(range(SIZE), range(SIZE)):
        i = ky * SIZE + kz
        for ci, kx in enumerate(KXORD):
            WA[i, :, 64 * ci:64 * ci + 64] = Ks[:, :128, kx, ky, kz].T
        for g in range(2):
            for a in range(4):
                kx = 4 * g + a
                if kx > 6:
                    continue
                W2[i, 32 * a:32 * a + 32, 64 * g:64 * (g + 1)] = \
                    Ks[:, 128:160, kx, ky, kz].T
    return WA, W2


def _gam_bias(bn_g_s, bn_g_v, bias_s):
    """Per-channel gamma [64] (vector gammas replicated x3) and bias [64]."""
    gam = np.empty(64, np.float32)
    gam[:16] = np.asarray(bn_g_s, np.float32)
    gam[16:] = np.repeat(np.asarray(bn_g_v, np.float32), 3)
    bias = np.zeros(64, np.float32)
    bias[:16] = np.asarray(bias_s, np.float32)
    return gam, bias


# ---------------------------------------------------------------- matmul plan

def _box(ky, kz):
    """Valid output range + slab coords for kernel offsets (ky, kz)."""
    d = kz - 3
    p = d % 2
    zofs = (d - p) // 2
    oy0 = max(0, (4 - ky) // 2)
    oy1 = min(16, (34 - ky) // 2 + 1)
    iy0 = 2 * oy0 + ky - 3
    return dict(p=p, zs=zofs + 2, iy0=iy0, oyc=oy1 - oy0, oy0=oy0)


def _mm_plan():
    """Matmul descriptors (src, i, wc, ww, x0, bank) in issue order.

    Each matmul writes one full psum bank = slots (slot0, slot0+1):
    partitions 0:ww x 2 x [oy0:oy0+oyc] x 16 (the ISA caps a matmul's
    output at one 2KB psum bank). Slot s holds plane s at
    partitions 0:64 (L) and plane s-1 at 64:128 (U). Chunk-2 c2p uses slots
    10..15 (banks 5-7): slot 10+j-2 L = plane j lower-taps, slot 10+j U =
    plane j upper-taps. Slot 8 L / slot 9 / U(0) are garbage sinks.

    Order: chunk-2 first (its half-size slab loads fast and its compute
    covers the big chunk-1 slab's DMA), then chunk-1.
    """
    plan = []
    for i in range(49):
        plan.append(('c2', i, 0, 64, 0, 0, 2))      # c2s g0 -> L(0),L(1)
        plan.append(('c2', i, 0, 128, 2, 10, 2))    # c2p -> slots (10,11)
        plan.append(('c2', i, 0, 128, 4, 12, 2))    # slots (12,13)
        plan.append(('c2', i, 0, 128, 6, 14, 2))    # slots (14,15)
        plan.append(('c2', i, 64, 64, 8, 6, 2))     # c2s g1 -> L(6),L(7)
    for i in range(49):
        for wc, ka in ((0, 0), (128, 4), (256, 1)):  # P(0,2), P(4,6), P(1,3)
            for s0 in (0, 2, 4, 6, 8):
                plan.append(('c1', i, wc, 128, ka + 2 * s0, s0, 2))
        for s0 in (0, 2, 4, 6):                      # S5, L only
            plan.append(('c1', i, 384, 64, 5 + 2 * s0, s0, 2))
    return plan


_PLAN = _mm_plan()

# stop flags: last matmul touching each psum bank (each such matmul must be
# the last toucher of EVERY bank it spans, which this plan order guarantees)
_LAST_IDX = {}
for _n, _d in enumerate(_PLAN):
    s0, npos = _d[5], _d[6]
    for _k in range(npos):
        _LAST_IDX[(s0 + _k) // 2] = _n
_STOPS = set(_LAST_IDX.values())
for _b, _n in _LAST_IDX.items():
    _s0, _np = _PLAN[_n][5], _PLAN[_n][6]
    for _k in range(_np):
        assert _LAST_IDX[(_s0 + _k) // 2] == _n, (_b, _n)


# ---------------------------------------------------------------- numpy shadow

def _shadow_core(c1, c2e, WA, W2):
    """Execute the matmul plan in numpy. Returns conv output [64, 8, 16, 16]."""
    slots = np.zeros((16, 128, 16, 16), np.float32)
    for src, i, wc, ww, x0, s0, npos in _PLAN:
        ky, kz = i // 7, i % 7
        bx = _box(ky, kz)
        sl = c2e if src == 'c2' else c1
        step = 1 if src == 'c2' else 2
        rhs = sl[:, x0:x0 + (npos - 1) * step + 1:step,
                 bx['iy0']:bx['iy0'] + 2 * bx['oyc']:2, bx['p'],
                 bx['zs']:bx['zs'] + 16]
        lhsT = (W2 if src == 'c2' else WA)[i][:, wc:wc + ww]
        contrib = np.einsum('km,kpbc->mpbc', lhsT, rhs)
        for k in range(npos):
            slots[s0 + k][:ww, bx['oy0']:bx['oy0'] + bx['oyc'], :] += contrib[:, k]
    out = np.empty((OXC, CO, 16, 16), np.float32)
    for j in range(OXC):
        acc = slots[j][0:64].copy()                    # L(j)
        acc = acc + slots[j + 1][64:128]               # U(j+1)
        if j >= 2:
            acc = acc + slots[8 + j][0:64]             # LB: slot 10+j-2
        if j <= 5:
            acc = acc + slots[10 + j][64:128]          # UB: slot 10+j
        out[j] = acc
    return out.transpose(1, 0, 2, 3)


def shadow_forward(inp):
    """Full-model numpy shadow of the device computation (for plan validation)."""
    svt = _svt_sym(inp['sv'])
    Ks = _assemble_kernel_sym(inp)
    WA, W2 = _weight_slabs(Ks)
    gam, bias = _gam_bias(inp['bn_g_s'], inp['bn_g_v'], inp['bias_s'])

    y = np.zeros((B, CO, 16, 16, 16), np.float32)
    ss = np.zeros(64, np.float64)
    for c in range(NCORES):
        b, h = c // 2, c % 2
        c1, c2e = _core_slabs(svt, b, h)
        out = _shadow_core(c1, c2e, WA, W2)
        y[b, :, 8 * h:8 * h + 8] = out
        ss += (out.astype(np.float64) ** 2).sum(axis=(1, 2, 3))

    var = np.empty(64)
    var[:16] = ss[:16] * VAR_S_DIV
    vv = (ss[16::3] + ss[17::3] + ss[18::3]) * VAR_V_DIV
    var[16:] = np.repeat(vv, 3)
    scale = gam / np.sqrt(var + EPS)
    y = y * scale[None, :, None, None, None].astype(np.float32)
    y[:, :16] = np.maximum(y[:, :16] + bias[:16][None, :, None, None, None], 0.0)
    return y


# ---------------------------------------------------------------- bass kernel

_CACHED = {}


def _build_bass():
    import concourse.bass as bass
    import concourse.tile as tile
    import concourse.mybir as mybir
    from concourse import bacc

    f32 = mybir.dt.float32
    f32r = mybir.dt.float32r
    bf16 = mybir.dt.bfloat16

    nc = bacc.Bacc("TRN2", target_bir_lowering=False, debug=False, num_devices=NCORES)

    in1 = nc.dram_tensor("in1", list(SLAB1_SHAPE), f32r, kind="ExternalInput").ap()
    in2e = nc.dram_tensor("in2e", list(SLAB2_SHAPE), f32r, kind="ExternalInput").ap()
    wa_in = nc.dram_tensor("wa_in", [49, 128, WA_COLS], f32r, kind="ExternalInput").ap()
    w2_in = nc.dram_tensor("w2_in", [49, 128, W2_COLS], f32r, kind="ExternalInput").ap()
    gam_in = nc.dram_tensor("gam_in", [64, 1], f32, kind="ExternalInput").ap()
    bias_in = nc.dram_tensor("bias_in", [64, 1], f32, kind="ExternalInput").ap()
    out_d = nc.dram_tensor("out", [CO, OXC, 16, 16], f32, kind="ExternalOutput").ap()

    with tile.TileContext(nc) as tc:
        with (
            tc.tile_pool(name="slab", bufs=1) as slab_pool,
            tc.tile_pool(name="wp", bufs=4) as wpool,
            tc.tile_pool(name="ps", bufs=1, space="PSUM") as ps,
            tc.tile_pool(name="outp", bufs=1) as outp,
            tc.tile_pool(name="stat", bufs=1) as stat,
            tc.tile_pool(name="dram", bufs=1, space="DRAM") as dram,
        ):
            # 8 psum banks = 16 half-bank slots (garbage: 8L, 9, U(0)),
            # grouped as 4 two-bank tiles so one matmul can cover 4 slots
            qt = [ps.tile([128, 4, 16, 16], f32, tag=f"qt{t}", name=f"qt{t}")
                  for t in range(4)]

            def slot(s):
                return qt[s // 4][:, s % 4]

            # tiny BN params first on the SP queue so they're resident early
            gam_t = stat.tile([CO, 1], f32, tag="gam")
            bias_t = stat.tile([CO, 1], f32, tag="bias")
            nc.sync.dma_start(gam_t[:], gam_in[:])
            nc.sync.dma_start(bias_t[:], bias_in[:])

            # slabs in bf16 (fp32r weights keep the contraction accurate; the
            # halved moving-operand footprint relieves SBUF read pressure and
            # halves slab DMA). Split across both HWDGE rings: SP carries the
            # early chunk-2 planes + chunk-1, Activation the late c2 planes.
            sl2 = slab_pool.tile(list(SLAB2_SHAPE), f32r, tag="slab2",
                                 name="slab_c2")
            sl1 = slab_pool.tile(list(SLAB1_SHAPE), f32r, tag="slab",
                                 name="slab_c1")
            nc.sync.dma_start(sl2[:, 0:2], in2e[:, 0:2])
            nc.sync.dma_start(sl2[:, 2:6], in2e[:, 2:6])
            nc.sync.dma_start(sl2[:, 6:10], in2e[:, 6:10])
            nc.sync.dma_start(sl1[:, 0:12], in1[:, 0:12])
            nc.sync.dma_start(sl1[:, 12:NXS], in1[:, 12:NXS])

            # start=True clears the WHOLE psum bank, so open each bank once
            # with a zero-weight full-bank matmul (also a WAW dep that orders
            # it before every accumulate); all real matmuls use start=False.
            # rhs is a memset zeros tile so the opens run before any DMA lands.
            zw_f = stat.tile([128, 128], f32, tag="zw")
            zr_f = stat.tile([128, 512], f32, tag="zr")
            nc.vector.memset(zw_f[:], 0.0)
            nc.vector.memset(zr_f[:], 0.0)
            zw = zw_f.bitcast(f32r)
            zr = zr_f.bitcast(f32r)
            for b in range(8):
                nc.tensor.matmul(qt[b // 2][:, 2 * (b % 2):2 * (b % 2) + 2],
                                 zw[:], zr[:], start=True, stop=False)

            # chunk-2 weights via gpsimd SWDGE (small tiles, keeps the fast
            # SP HWDGE ring free for slabs), chunk-1 weights on SP behind the
            # slabs (they are consumed only after the chunk-2 phase)
            w2t = {}
            for i in range(49):
                w = wpool.tile([128, W2_COLS], f32r, tag="w2", name=f"w2_{i}",
                               bufs=8)
                nc.gpsimd.dma_start(w[:], w2_in[i])
                w2t[i] = w
            wat = {}
            for i in range(49):
                w = wpool.tile([128, WA_COLS], f32r, tag="wa", name=f"wa_{i}",
                               bufs=4)
                nc.sync.dma_start(w[:], wa_in[i])
                wat[i] = w

            # warm up the collective engine early so the real AllReduce at the
            # tail skips the ~11us first-use wake latency (result discarded)
            cc_w_in = dram.tile([1, 64], f32, tag="ccwi")
            cc_w_out = dram.tile([1, 64], f32, tag="ccwo")
            nc.gpsimd.collective_compute(
                "AllReduce", mybir.AluOpType.add,
                replica_groups=[list(range(NCORES))],
                ins=[cc_w_in.opt()], outs=[cc_w_out.opt()],
            )

            for n, (src, i, wc, ww, x0, s0, npos) in enumerate(_PLAN):
                ky, kz = i // 7, i % 7
                bx = _box(ky, kz)
                if src == 'c2':
                    w, sl, step = w2t[i], sl2, 1
                else:
                    w, sl, step = wat[i], sl1, 2
                rhs = sl[:, x0:x0 + (npos - 1) * step + 1:step,
                         bx['iy0']:bx['iy0'] + 2 * bx['oyc'] - 1:2, bx['p'],
                         bx['zs']:bx['zs'] + 16]
                p0 = s0 % 4
                out_ap = qt[s0 // 4][0:ww, p0:p0 + npos,
                                     bx['oy0']:bx['oy0'] + bx['oyc'], :]
                nc.tensor.matmul(out_ap, w[:, wc:wc + ww], rhs,
                                 start=False, stop=n in _STOPS)

            # rendezvous: a 4-byte token dependent on the first evacuated
            # plane enters a dummy AllReduce, so the cores align while the
            # rest of the evac/stats chain runs; the real AllReduce below
            # then finds every peer already caught up
            cc_b_in = dram.tile([1, 64], f32, tag="ccbi")
            cc_b_out = dram.tile([1, 64], f32, tag="ccbo")

            # evacuate: plane j = L(j) + U(j+1) [+ LB(10+j-2) j>=2]
            #                                   [+ UB(10+j)   j<=5]
            # a TensorTensor may read only ONE psum operand: Scalar engine
            # copies psum->sbuf, Vector accumulates the second psum operand
            osb = outp.tile([CO, OXC, 16, 16], f32, tag="osb")
            usb = outp.tile([128, OXC, 16, 16], f32, tag="usb")
            for j in range(OXC):
                nc.scalar.activation(osb[:, j], slot(j)[0:64],
                                     mybir.ActivationFunctionType.Copy,
                                     scale=1.0)
                if j >= 2:
                    nc.vector.tensor_add(osb[:, j], osb[:, j],
                                         slot(8 + j)[0:64])
                nc.scalar.activation(usb[64:128, j], slot(j + 1)[64:128],
                                     mybir.ActivationFunctionType.Copy,
                                     scale=1.0)
                if j <= 5:
                    nc.vector.tensor_add(usb[64:128, j], usb[64:128, j],
                                         slot(10 + j)[64:128])
            nc.sync.dma_start(cc_b_in[0:1, 0:1], osb[0:1, 0, 0, 0:1])
            nc.gpsimd.collective_compute(
                "AllReduce", mybir.AluOpType.add,
                replica_groups=[list(range(NCORES))],
                ins=[cc_b_in.opt()], outs=[cc_b_out.opt()],
            )

            # move upper-half partials down to partitions 0:64 and add
            nc.sync.dma_start(usb[0:64], usb[64:128])
            of = osb.rearrange("c x y z -> c (x y z)")
            uf = usb.rearrange("c x y z -> c (x y z)")
            nc.vector.tensor_add(of[:, :], of[:, :], uf[0:64, :])

            # per-channel sum of squares in ONE scalar-engine op (Square with
            # free-axis accumulator) -> local variance contribution
            # (linear in the sums, so the AllReduce can carry variance
            # directly and the post-collective chain stays short)
            sq = outp.tile([CO, 2048], f32, tag="sq")
            ssq = stat.tile([CO, 1], f32, tag="ssq")
            nc.scalar.activation(sq[:], of[:, :],
                                 mybir.ActivationFunctionType.Square,
                                 scale=1.0, accum_out=ssq[:, :])
            ss_row = stat.tile([1, 64], f32, tag="ssrow")
            vloc = stat.tile([1, 64], f32, tag="vloc")
            tmp16 = stat.tile([1, 16], f32, tag="tmp16")
            ss_dram = dram.tile([1, 64], f32, tag="ssd")
            nc.sync.dma_start(ss_dram[0, :], ssq[:, 0])
            nc.sync.dma_start(ss_row[:], ss_dram[:])
            nc.vector.tensor_add(tmp16[:], ss_row[:, 16::3], ss_row[:, 17::3])
            nc.vector.tensor_add(tmp16[:], tmp16[:], ss_row[:, 18::3])
            nc.vector.tensor_scalar_mul(vloc[:, 0:16], ss_row[:, 0:16], VAR_S_DIV)
            for j in range(3):
                nc.vector.tensor_scalar_mul(vloc[:, 16 + j::3], tmp16[:], VAR_V_DIV)

            v_dram = dram.tile([1, 64], f32, tag="vd")
            v_red = dram.tile([1, 64], f32, tag="vr")
            nc.sync.dma_start(v_dram[:], vloc[:])
            nc.gpsimd.collective_compute(
                "AllReduce", mybir.AluOpType.add,
                replica_groups=[list(range(NCORES))],
                ins=[v_dram.opt()], outs=[v_red.opt()],
            )

            # scale = gamma / sqrt(var + eps), in per-partition layout
            var_col = stat.tile([CO, 1], f32, tag="varcol")
            nc.sync.dma_start(var_col[:, 0], v_red[0, :])
            eps_t = stat.tile([CO, 1], f32, tag="eps")
            nc.vector.memset(eps_t[:], EPS)
            sd = stat.tile([CO, 1], f32, tag="sd")
            nc.scalar.activation(sd[:], var_col[:], mybir.ActivationFunctionType.Sqrt,
                                 bias=eps_t[:], scale=1.0)
            inv = stat.tile([CO, 1], f32, tag="inv")
            nc.vector.reciprocal(inv[:], sd[:])
            scale_col = stat.tile([CO, 1], f32, tag="sccol")
            nc.vector.tensor_mul(scale_col[:], inv[:], gam_t[:])

            # apply BN scale everywhere, then bias+relu on scalar channels
            nc.vector.tensor_scalar_mul(of[:, :], of[:, :], scale_col[:, :])
            nc.scalar.activation(of[0:16, :], of[0:16, :],
                                 mybir.ActivationFunctionType.Relu,
                                 bias=bias_t[0:16, :], scale=1.0)
            nc.sync.dma_start(out_d[:], osb[:])

    nc.compile()
    return nc


def _install_ntff_hook():
    import sys, types
    if "antenv.axon_hooks" in sys.modules:
        return
    mod = types.ModuleType("antenv.axon_hooks")
    mod._hook = None
    mod.set_axon_ntff_profile_hook = lambda h: setattr(mod, "_hook", h)
    mod.get_axon_ntff_profile_hook = lambda: mod._hook
    sys.modules["antenv.axon_hooks"] = mod
    try:
        import antenv
        antenv.axon_hooks = mod
        from trn_agent_boot.trn_boot import _ntff_profile_via_ctypes
        mod.set_axon_ntff_profile_hook(_ntff_profile_via_ctypes("/opt/axon/libaxon_pjrt.so"))
    except Exception:
        pass


def run_on_hw(inp, trace=False):
    """Run the kernel on 8 cores. Returns (full output [4,64,16,16,16], results)."""
    from concourse.bass_utils import run_bass_kernel_spmd

    if "nc" not in _CACHED:
        _install_ntff_hook()
        _CACHED["nc"] = _build_bass()
    nc = _CACHED["nc"]

    svt = _svt_sym(inp['sv'])
    Ks = _assemble_kernel_sym(inp)
    WA, W2 = _weight_slabs(Ks)
    gam, bias = _gam_bias(inp['bn_g_s'], inp['bn_g_v'], inp['bias_s'])

    in_maps = []
    for c in range(NCORES):
        b, h = c // 2, c % 2
        c1, c2e = _core_slabs(svt, b, h)
        in_maps.append({
            "in1": c1,
            "in2e": c2e,
            "wa_in": WA, "w2_in": W2,
            "gam_in": gam.reshape(64, 1),
            "bias_in": bias.reshape(64, 1),
        })

    res = run_bass_kernel_spmd(nc, in_maps, core_ids=list(range(NCORES)), trace=trace)

    y = np.zeros((B, CO, 16, 16, 16), np.float32)
    for c in range(NCORES):
        b, h = c // 2, c % 2
        y[b, :, 8 * h:8 * h + 8] = res.results[c]["out"]
    return y, res


def kernel(**inputs) -> np.ndarray:
    y, _ = run_on_hw(inputs, trace=False)
    return y


# revision 15
# speedup vs baseline: 1.3187x; 1.3187x over previous
"""SE(3)-CNN block (TensorProduct -> SE3Conv -> SE3BatchNorm -> BiasRelu) on 8 trn2 cores.

Sharding: core c = (batch b=c//2, out-x-half h=c%2). Each core computes all 64
output channels for 8 of 16 output x-planes of one batch; per-field BN second
moments are combined with a tiny [1,64] AllReduce across all 8 cores.

v2 conv strategy (vs single-plane baseline):
- Every fp32r matmul covers TWO output x-planes = one full psum bank, free
  dim 448-512 (>= 256 keeps fp32r in its fast streaming mode) and half the
  instruction count.
- No M=64 boundary singles: each chunk-1 kx pair block runs its full slot
  range (s=0..9); out-of-range tap contributions land in psum half-banks the
  evacuation never reads (slot 8 L, slot 9) fed from x-padded slab planes.
- chunk2 (channels 128:160, 4 kx taps packed per 128-row contraction) uses
  slots 10-15 (banks 5-7) plus L(0),L(1),L(6),L(7).
- Weight slab dedup: chunk-1 kx blocks stored once in column order
  [0,2 | 4,6 | 1,3 | 5] so every used pair is contiguous (11.2MB vs 24MB).
- Weight DMA on the Activation HWDGE queue, slabs + tail/stat DMAs on SP,
  evacuation adds split across Vector and GpSimd.
"""
import numpy as np
from itertools import product

# problem constants (from spec / reference)
B = 4
S_IN = 16
V_IN = 16
CO = 64          # 16 scalar + 48 vector output channels
CI = 160         # 16 s + 48 v + 96 t_sym
SIZE = 7
PAD = 3
STRIDE = 2
EPS = 1e-5
NCORES = 8
NXS = 23         # x-padded slab planes per core (px 22 feeds garbage slots only)
NZS = 19         # z-padded: zi_slab = zi_global + 2, covering zofs in [-2, 1]
NX2 = 10         # chunk-2 even slab planes (px = 2*xi, xi 0..9)
OXC = 8          # out x-planes per core
PAIRS = [(0, 0), (0, 1), (0, 2), (1, 1), (1, 2), (2, 2)]
VAR_S_DIV = 1.0 / (B * 16 * 16 * 16)
VAR_V_DIV = 1.0 / (B * 3 * 16 * 16 * 16)

SLAB1_SHAPE = (128, NXS, 32, 2, NZS)   # [ci, px, iy, pz, zi]
SLAB2_SHAPE = (128, NX2, 32, 2, NZS)
WA_COLS = 448    # [k0|k2 | k4|k6 | k1|k3 | k5]
W2_COLS = 2 * CO


# ---------------------------------------------------------------- host prep

def _assemble_kernel_sym(inp):
    """Assemble the dense conv kernel [64, 208, 7,7,7] and symmetrize the
    t-block -> [64, 160, 7,7,7]."""
    def blk(w, basis):
        w = np.asarray(w, np.float32)
        basis = np.asarray(basis, np.float32)
        mo, mi, nb = w.shape
        do, di = basis.shape[1], basis.shape[2]
        k = np.einsum('uvb,bijxyz->uivjxyz', w, basis)
        return k.reshape(mo * do, mi * di, SIZE, SIZE, SIZE)

    row_s = np.concatenate([blk(inp['w_ss'], inp['basis_ss']),
                            blk(inp['w_sv'], inp['basis_sv']),
                            blk(inp['w_st'], inp['basis_st'])], axis=1)
    row_v = np.concatenate([blk(inp['w_vs'], inp['basis_vs']),
                            blk(inp['w_vv'], inp['basis_vv']),
                            blk(inp['w_vt'], inp['basis_vt'])], axis=1)
    K = np.concatenate([row_s, row_v], axis=0)  # [64, 208, 7,7,7]

    Ks = np.empty((CO, CI, SIZE, SIZE, SIZE), np.float32)
    Ks[:, :64] = K[:, :64]
    for u in range(16):
        for pi, (i, j) in enumerate(PAIRS):
            src = K[:, 64 + 9 * u + 3 * i + j]
            if i != j:
                src = src + K[:, 64 + 9 * u + 3 * j + i]
            Ks[:, 64 + 6 * u + pi] = src
    return Ks


def _svt_sym(sv):
    """[4,64,32,32,32] -> symmetrized tensor-product features [4,160,32,32,32]."""
    sv = np.asarray(sv, np.float32)
    s = sv[:, :S_IN]
    v = sv[:, S_IN:].reshape(B, V_IN, 3, 32, 32, 32)
    t = np.empty((B, V_IN, 6, 32, 32, 32), np.float32)
    for pi, (i, j) in enumerate(PAIRS):
        t[:, :, pi] = v[:, :, i] * v[:, :, j]
    return np.concatenate([s, v.reshape(B, 48, 32, 32, 32),
                           t.reshape(B, 96, 32, 32, 32)], axis=1)


def _core_slabs(svt, b, h):
    """x/z zero-padded, z-parity-split slabs for core (b, h).

    c1 [128, 23, 32, 2, 19]: plane px holds global ix = px + 16h - 3.
    c2e [128, 10, 32, 2, 19]: block a (rows 32a:32a+32) of plane xi holds
    chunk-2 channels at ix = 2*xi + a + 16h - 3. zi_slab = zi_global + 2.
    """
    sp = svt[b].reshape(CI, 32, 32, 16, 2)   # (ci, x, y, zi, pz); iz = 2*zi + pz
    sp = np.moveaxis(sp, 4, 3)               # (ci, x, y, pz, zi)
    x0 = 16 * h - 3
    c1 = np.zeros(SLAB1_SHAPE, np.float32)
    lo, hi = max(0, x0), min(32, x0 + NXS)
    c1[:, lo - x0:hi - x0, :, :, 2:18] = sp[:128, lo:hi]
    c2e = np.zeros(SLAB2_SHAPE, np.float32)
    for a in range(4):
        for xi in range(NX2):
            ix = 2 * xi + a + x0
            if 0 <= ix < 32:
                c2e[32 * a:32 * a + 32, xi, :, :, 2:18] = sp[128:160, ix]
    return c1, c2e


def _weight_slabs(Ks):
    """(WA, W2). WA [49, 128, 448]: chunk-1 taps in column order
    [k0|k2|k4|k6|k1|k3|k5] (64 cols each); pairs (0,2),(4,6),(1,3) are the
    contiguous 128-col windows at 0, 128, 256; k5 singles at 384.
    W2 [49, 128, 128]: 4-way kx-merged chunk-2 (g=0: kx 0..3 lower 64 cols,
    g=1: kx 4..6 upper)."""
    KXORD = [0, 2, 4, 6, 1, 3, 5]
    WA = np.zeros((49, 128, WA_COLS), np.float32)
    W2 = np.zeros((49, 128, W2_COLS), np.float32)
    for ky, kz in product(range(SIZE), range(SIZE)):
        i = ky * SIZE + kz
        for ci, kx in enumerate(KXORD):
            WA[i, :, 64 * ci:64 * ci + 64] = Ks[:, :128, kx, ky, kz].T
        for g in range(2):
            for a in range(4):
                kx = 4 * g + a
                if kx > 6:
                    continue
                W2[i, 32 * a:32 * a + 32, 64 * g:64 * (g + 1)] = \
                    Ks[:, 128:160, kx, ky, kz].T
    return WA, W2


def _gam_bias(bn_g_s, bn_g_v, bias_s):
    """Per-channel gamma [64] (vector gammas replicated x3) and bias [64]."""
    gam = np.empty(64, np.float32)
    gam[:16] = np.asarray(bn_g_s, np.float32)
    gam[16:] = np.repeat(np.asarray(bn_g_v, np.float32), 3)
    bias = np.zeros(64, np.float32)
    bias[:16] = np.asarray(bias_s, np.float32)
    return gam, bias


# ---------------------------------------------------------------- matmul plan

def _box(ky, kz):
    """Valid output range + slab coords for kernel offsets (ky, kz)."""
    d = kz - 3
    p = d % 2
    zofs = (d - p) // 2
    oy0 = max(0, (4 - ky) // 2)
    oy1 = min(16, (34 - ky) // 2 + 1)
    iy0 = 2 * oy0 + ky - 3
    return dict(p=p, zs=zofs + 2, iy0=iy0, oyc=oy1 - oy0, oy0=oy0)


def _mm_plan():
    """Matmul descriptors (src, i, wc, ww, x0, bank) in issue order.

    Each matmul writes one full psum bank `bank` = slots (2*bank, 2*bank+1):
    partitions 0:ww x 2 halves x [oy0:oy0+oyc] x 16. Slot s holds plane s at
    partitions 0:64 (L) and plane s-1 at 64:128 (U). Chunk-2 c2p uses slots
    10..15 (banks 5-7): slot 10+j-2 L = plane j lower-taps, slot 10+j U =
    plane j upper-taps. Slot 8 L / slot 9 / U(0) are garbage sinks.

    Order: chunk-2 first (its half-size slab loads fast and its compute
    covers the big chunk-1 slab's DMA), then chunk-1.
    """
    plan = []
    for i in range(49):
        plan.append(('c2', i, 0, 64, 0, 0))     # c2s g0 -> L(0),L(1)
        plan.append(('c2', i, 0, 128, 2, 5))    # c2p -> slots (10,11)
        plan.append(('c2', i, 0, 128, 4, 6))    # slots (12,13)
        plan.append(('c2', i, 0, 128, 6, 7))    # slots (14,15)
        plan.append(('c2', i, 64, 64, 8, 3))    # c2s g1 -> L(6),L(7)
    for i in range(49):
        for bank in range(5):                   # P(0,2): px (4b, 4b+2)
            plan.append(('c1', i, 0, 128, 4 * bank, bank))
        for bank in range(5):                   # P(4,6): px (4+4b, 6+4b)
            plan.append(('c1', i, 128, 128, 4 + 4 * bank, bank))
        for bank in range(5):                   # P(1,3): px (1+4b, 3+4b)
            plan.append(('c1', i, 256, 128, 1 + 4 * bank, bank))
        for bank in range(4):                   # S5: px (5+4b, 7+4b), L only
            plan.append(('c1', i, 384, 64, 5 + 4 * bank, bank))
    return plan


_PLAN = _mm_plan()

# stop flags: last matmul touching each psum bank
_LAST_IDX = {}
for _n, _d in enumerate(_PLAN):
    _LAST_IDX[_d[5]] = _n
_STOPS = set(_LAST_IDX.values())


# ---------------------------------------------------------------- numpy shadow

def _shadow_core(c1, c2e, WA, W2):
    """Execute the matmul plan in numpy. Returns conv output [64, 8, 16, 16]."""
    banks = np.zeros((8, 128, 2, 16, 16), np.float32)
    for src, i, wc, ww, x0, bank in _PLAN:
        ky, kz = i // 7, i % 7
        bx = _box(ky, kz)
        sl = c2e if src == 'c2' else c1
        step = 1 if src == 'c2' else 2
        rhs = sl[:, x0:x0 + step + 1:step,
                 bx['iy0']:bx['iy0'] + 2 * bx['oyc']:2, bx['p'],
                 bx['zs']:bx['zs'] + 16]
        lhsT = (W2 if src == 'c2' else WA)[i][:, wc:wc + ww]
        contrib = np.einsum('km,kpbc->mpbc', lhsT, rhs)
        banks[bank][:ww, :, bx['oy0']:bx['oy0'] + bx['oyc'], :] += contrib
    out = np.empty((OXC, CO, 16, 16), np.float32)
    for j in range(OXC):
        acc = banks[j // 2][0:64, j % 2].copy()        # L(j)
        su = j + 1
        acc = acc + banks[su // 2][64:128, su % 2]     # U(j+1)
        if j >= 2:
            s = 8 + j                                  # LB: slot 10+j-2
            acc = acc + banks[s // 2][0:64, s % 2]
        if j <= 5:
            s = 10 + j                                 # UB: slot 10+j
            acc = acc + banks[s // 2][64:128, s % 2]
        out[j] = acc
    return out.transpose(1, 0, 2, 3)


def shadow_forward(inp):
    """Full-model numpy shadow of the device computation (for plan validation)."""
    svt = _svt_sym(inp['sv'])
    Ks = _assemble_kernel_sym(inp)
    WA, W2 = _weight_slabs(Ks)
    gam, bias = _gam_bias(inp['bn_g_s'], inp['bn_g_v'], inp['bias_s'])

    y = np.zeros((B, CO, 16, 16, 16), np.float32)
    ss = np.zeros(64, np.float64)
    for c in range(NCORES):
        b, h = c // 2, c % 2
        c1, c2e = _core_slabs(svt, b, h)
        out = _shadow_core(c1, c2e, WA, W2)
        y[b, :, 8 * h:8 * h + 8] = out
        ss += (out.astype(np.float64) ** 2).sum(axis=(1, 2, 3))

    var = np.empty(64)
    var[:16] = ss[:16] * VAR_S_DIV
    vv = (ss[16::3] + ss[17::3] + ss[18::3]) * VAR_V_DIV
    var[16:] = np.repeat(vv, 3)
    scale = gam / np.sqrt(var + EPS)
    y = y * scale[None, :, None, None, None].astype(np.float32)
    y[:, :16] = np.maximum(y[:, :16] + bias[:16][None, :, None, None, None], 0.0)
    return y


# ---------------------------------------------------------------- bass kernel

_CACHED = {}


def _build_bass():
    import concourse.bass as bass
    import concourse.tile as tile
    import concourse.mybir as mybir
    from concourse import bacc

    f32 = mybir.dt.float32
    f32r = mybir.dt.float32r
    bf16 = mybir.dt.bfloat16

    nc = bacc.Bacc("TRN2", target_bir_lowering=False, debug=False, num_devices=NCORES)

    in1 = nc.dram_tensor("in1", list(SLAB1_SHAPE), f32r, kind="ExternalInput").ap()
    in2e = nc.dram_tensor("in2e", list(SLAB2_SHAPE), f32r, kind="ExternalInput").ap()
    wa_in = nc.dram_tensor("wa_in", [49, 128, WA_COLS], f32r, kind="ExternalInput").ap()
    w2_in = nc.dram_tensor("w2_in", [49, 128, W2_COLS], f32r, kind="ExternalInput").ap()
    gam_in = nc.dram_tensor("gam_in", [64, 1], f32, kind="ExternalInput").ap()
    bias_in = nc.dram_tensor("bias_in", [64, 1], f32, kind="ExternalInput").ap()
    out_d = nc.dram_tensor("out", [CO, OXC, 16, 16], f32, kind="ExternalOutput").ap()

    with tile.TileContext(nc) as tc:
        with (
            tc.tile_pool(name="slab", bufs=1) as slab_pool,
            tc.tile_pool(name="wp", bufs=4) as wpool,
            tc.tile_pool(name="ps", bufs=1, space="PSUM") as ps,
            tc.tile_pool(name="outp", bufs=1) as outp,
            tc.tile_pool(name="stat", bufs=1) as stat,
            tc.tile_pool(name="dram", bufs=1, space="DRAM") as dram,
        ):
            # 8 psum banks = 16 half-bank slots (garbage: 8L, 9, U(0))
            pq = [ps.tile([128, 2, 16, 16], f32, tag=f"pq{t}", name=f"pq{t}")
                  for t in range(8)]

            # tiny BN params first on the SP queue so they're resident early
            gam_t = stat.tile([CO, 1], f32, tag="gam")
            bias_t = stat.tile([CO, 1], f32, tag="bias")
            nc.sync.dma_start(gam_t[:], gam_in[:])
            nc.sync.dma_start(bias_t[:], bias_in[:])

            # slabs in bf16 (fp32r weights keep the contraction accurate; the
            # halved moving-operand footprint relieves SBUF read pressure and
            # halves slab DMA). Split across both HWDGE rings: SP carries the
            # early chunk-2 planes + chunk-1, Activation the late c2 planes.
            sl2 = slab_pool.tile(list(SLAB2_SHAPE), f32r, tag="slab2",
                                 name="slab_c2")
            sl1 = slab_pool.tile(list(SLAB1_SHAPE), f32r, tag="slab",
                                 name="slab_c1")
            nc.sync.dma_start(sl2[:, 0:2], in2e[:, 0:2])
            nc.sync.dma_start(sl2[:, 2:6], in2e[:, 2:6])
            nc.scalar.dma_start(sl2[:, 6:10], in2e[:, 6:10])
            nc.sync.dma_start(sl1[:, 0:12], in1[:, 0:12])
            nc.sync.dma_start(sl1[:, 12:NXS], in1[:, 12:NXS])

            # start=True clears the WHOLE psum bank, so open each bank once
            # with a zero-weight full-bank matmul (also a WAW dep that orders
            # it before every accumulate); all real matmuls use start=False.
            # rhs is a memset zeros tile so the opens run before any DMA lands.
            zw_f = stat.tile([128, 128], f32, tag="zw")
            zr_f = stat.tile([128, 512], f32, tag="zr")
            nc.vector.memset(zw_f[:], 0.0)
            nc.vector.memset(zr_f[:], 0.0)
            zw = zw_f.bitcast(f32r)
            zr = zr_f.bitcast(f32r)
            for t in range(8):
                nc.tensor.matmul(pq[t].rearrange("c a y z -> c (a y z)"),
                                 zw[:], zr[:], start=True, stop=False)

            # weights on the Activation HWDGE queue, one DMA per tile
            w2t = {}
            for i in range(49):
                w = wpool.tile([128, W2_COLS], f32r, tag="w2", name=f"w2_{i}",
                               bufs=4)
                nc.scalar.dma_start(w[:], w2_in[i])
                w2t[i] = w
            wat = {}
            for i in range(49):
                w = wpool.tile([128, WA_COLS], f32r, tag="wa", name=f"wa_{i}",
                               bufs=4)
                nc.scalar.dma_start(w[:], wa_in[i])
                wat[i] = w

            for n, (src, i, wc, ww, x0, bank) in enumerate(_PLAN):
                ky, kz = i // 7, i % 7
                bx = _box(ky, kz)
                if src == 'c2':
                    w, sl, step = w2t[i], sl2, 1
                else:
                    w, sl, step = wat[i], sl1, 2
                rhs = sl[:, x0:x0 + step + 1:step,
                         bx['iy0']:bx['iy0'] + 2 * bx['oyc'] - 1:2, bx['p'],
                         bx['zs']:bx['zs'] + 16]
                out_ap = pq[bank][0:ww, 0:2, bx['oy0']:bx['oy0'] + bx['oyc'], :]
                nc.tensor.matmul(out_ap, w[:, wc:wc + ww], rhs,
                                 start=False, stop=n in _STOPS)

            # evacuate: plane j = L(j) + U(j+1) [+ LB(10+j-2) j>=2]
            #                                   [+ UB(10+j)   j<=5]
            # a TensorTensor may read only ONE psum operand: Scalar engine
            # copies psum->sbuf, Vector accumulates the second psum operand
            osb = outp.tile([CO, OXC, 16, 16], f32, tag="osb")
            usb = outp.tile([128, OXC, 16, 16], f32, tag="usb")
            for j in range(OXC):
                nc.scalar.activation(osb[:, j], pq[j // 2][0:64, j % 2],
                                     mybir.ActivationFunctionType.Copy,
                                     scale=1.0)
                if j >= 2:
                    s = 8 + j
                    nc.vector.tensor_add(osb[:, j], osb[:, j],
                                         pq[s // 2][0:64, s % 2])
                su = j + 1
                nc.scalar.activation(usb[64:128, j],
                                     pq[su // 2][64:128, su % 2],
                                     mybir.ActivationFunctionType.Copy,
                                     scale=1.0)
                if j <= 5:
                    s = 10 + j
                    nc.vector.tensor_add(usb[64:128, j], usb[64:128, j],
                                         pq[s // 2][64:128, s % 2])
            # move upper-half partials down to partitions 0:64 and add
            u_dram = dram.tile([64, OXC, 16, 16], f32, tag="ud")
            nc.sync.dma_start(u_dram[:], usb[64:128])
            nc.sync.dma_start(usb[0:64], u_dram[:])
            of = osb.rearrange("c x y z -> c (x y z)")
            uf = usb.rearrange("c x y z -> c (x y z)")
            nc.vector.tensor_add(of[:, :], of[:, :], uf[0:64, :])

            # per-channel sum of squares in ONE scalar-engine op (Square with
            # free-axis accumulator) -> local variance contribution
            # (linear in the sums, so the AllReduce can carry variance
            # directly and the post-collective chain stays short)
            sq = outp.tile([CO, 2048], f32, tag="sq")
            ssq = stat.tile([CO, 1], f32, tag="ssq")
            nc.scalar.activation(sq[:], of[:, :],
                                 mybir.ActivationFunctionType.Square,
                                 scale=1.0, accum_out=ssq[:, :])
            ss_row = stat.tile([1, 64], f32, tag="ssrow")
            vloc = stat.tile([1, 64], f32, tag="vloc")
            tmp16 = stat.tile([1, 16], f32, tag="tmp16")
            ss_dram = dram.tile([1, 64], f32, tag="ssd")
            nc.sync.dma_start(ss_dram[0, :], ssq[:, 0])
            nc.sync.dma_start(ss_row[:], ss_dram[:])
            nc.vector.tensor_add(tmp16[:], ss_row[:, 16::3], ss_row[:, 17::3])
            nc.vector.tensor_add(tmp16[:], tmp16[:], ss_row[:, 18::3])
            nc.vector.tensor_scalar_mul(vloc[:, 0:16], ss_row[:, 0:16], VAR_S_DIV)
            for j in range(3):
                nc.vector.tensor_scalar_mul(vloc[:, 16 + j::3], tmp16[:], VAR_V_DIV)

            v_dram = dram.tile([1, 64], f32, tag="vd")
            v_red = dram.tile([1, 64], f32, tag="vr")
            nc.sync.dma_start(v_dram[:], vloc[:])
            nc.gpsimd.collective_compute(
                "AllReduce", mybir.AluOpType.add,
                replica_groups=[list(range(NCORES))],
                ins=[v_dram.opt()], outs=[v_red.opt()],
            )

            # scale = gamma / sqrt(var + eps), in per-partition layout
            var_col = stat.tile([CO, 1], f32, tag="varcol")
            nc.sync.dma_start(var_col[:, 0], v_red[0, :])
            eps_t = stat.tile([CO, 1], f32, tag="eps")
            nc.vector.memset(eps_t[:], EPS)
            sd = stat.tile([CO, 1], f32, tag="sd")
            nc.scalar.activation(sd[:], var_col[:], mybir.ActivationFunctionType.Sqrt,
                                 bias=eps_t[:], scale=1.0)
            inv = stat.tile([CO, 1], f32, tag="inv")
            nc.vector.reciprocal(inv[:], sd[:])
            scale_col = stat.tile([CO, 1], f32, tag="sccol")
            nc.vector.tensor_mul(scale_col[:], inv[:], gam_t[:])

            # apply BN scale everywhere, then bias+relu on scalar channels
            nc.vector.tensor_scalar_mul(of[:, :], of[:, :], scale_col[:, :])
            nc.scalar.activation(of[0:16, :], of[0:16, :],
                                 mybir.ActivationFunctionType.Relu,
                                 bias=bias_t[0:16, :], scale=1.0)
            nc.sync.dma_start(out_d[:], osb[:])

    nc.compile()
    return nc


def _install_ntff_hook():
    import sys, types
    if "antenv.axon_hooks" in sys.modules:
        return
    mod = types.ModuleType("antenv.axon_hooks")
    mod._hook = None
    mod.set_axon_ntff_profile_hook = lambda h: setattr(mod, "_hook", h)
    mod.get_axon_ntff_profile_hook = lambda: mod._hook
    sys.modules["antenv.axon_hooks"] = mod
    try:
        import antenv
        antenv.axon_hooks = mod
        from trn_agent_boot.trn_boot import _ntff_profile_via_ctypes
        mod.set_axon_ntff_profile_hook(_ntff_profile_via_ctypes("/opt/axon/libaxon_pjrt.so"))
    except Exception:
        pass


def run_on_hw(inp, trace=False):
    """Run the kernel on 8 cores. Returns (full output [4,64,16,16,16], results)."""
    from concourse.bass_utils import run_bass_kernel_spmd

    if "nc" not in _CACHED:
        _install_ntff_hook()
        _CACHED["nc"] = _build_bass()
    nc = _CACHED["nc"]

    svt = _svt_sym(inp['sv'])
    Ks = _assemble_kernel_sym(inp)
    WA, W2 = _weight_slabs(Ks)
    gam, bias = _gam_bias(inp['bn_g_s'], inp['bn_g_v'], inp['bias_s'])

    in_maps = []
    for c in range(NCORES):
        b, h = c // 2, c % 2
        c1, c2e = _core_slabs(svt, b, h)
        in_maps.append({
            "in1": c1,
            "in2e": c2e,
            "wa_in": WA, "w2_in": W2,
            "gam_in": gam.reshape(64, 1),
            "bias_in": bias.reshape(64, 1),
        })

    res = run_bass_kernel_spmd(nc, in_maps, core_ids=list(range(NCORES)), trace=trace)

    y = np.zeros((B, CO, 16, 16, 16), np.float32)
    for c in range(NCORES):
        b, h = c // 2, c % 2
        y[b, :, 8 * h:8 * h + 8] = res.results[c]["out"]
    return y, res


def kernel(**inputs) -> np.ndarray:
    y, _ = run_on_hw(inputs, trace=False)
    return y


# revision 16
# speedup vs baseline: 1.3203x; 1.0012x over previous
"""SE(3)-CNN block (TensorProduct -> SE3Conv -> SE3BatchNorm -> BiasRelu) on 8 trn2 cores.

Sharding: core c = (batch b=c//2, out-x-half h=c%2). Each core computes all 64
output channels for 8 of 16 output x-planes of one batch; per-field BN second
moments are combined with a tiny [1,64] AllReduce across all 8 cores.

v2 conv strategy (vs single-plane baseline):
- Every fp32r matmul covers TWO output x-planes = one full psum bank, free
  dim 448-512 (>= 256 keeps fp32r in its fast streaming mode) and half the
  instruction count.
- No M=64 boundary singles: each chunk-1 kx pair block runs its full slot
  range (s=0..9); out-of-range tap contributions land in psum half-banks the
  evacuation never reads (slot 8 L, slot 9) fed from x-padded slab planes.
- chunk2 (channels 128:160, 4 kx taps packed per 128-row contraction) uses
  slots 10-15 (banks 5-7) plus L(0),L(1),L(6),L(7).
- Weight slab dedup: chunk-1 kx blocks stored once in column order
  [0,2 | 4,6 | 1,3 | 5] so every used pair is contiguous (11.2MB vs 24MB).
- Weight DMA on the Activation HWDGE queue, slabs + tail/stat DMAs on SP,
  evacuation adds split across Vector and GpSimd.
"""
import numpy as np
from itertools import product

# problem constants (from spec / reference)
B = 4
S_IN = 16
V_IN = 16
CO = 64          # 16 scalar + 48 vector output channels
CI = 160         # 16 s + 48 v + 96 t_sym
SIZE = 7
PAD = 3
STRIDE = 2
EPS = 1e-5
NCORES = 8
NXS = 21         # x-padded slab planes per core
NZS = 19         # z-padded: zi_slab = zi_global + 2, covering zofs in [-2, 1]
NX2 = 10         # chunk-2 even slab planes (px = 2*xi, xi 0..9)
OXC = 8          # out x-planes per core
PAIRS = [(0, 0), (0, 1), (0, 2), (1, 1), (1, 2), (2, 2)]
VAR_S_DIV = 1.0 / (B * 16 * 16 * 16)
VAR_V_DIV = 1.0 / (B * 3 * 16 * 16 * 16)

SLAB1_SHAPE = (128, NXS, 32, 2, NZS)   # [ci, px, iy, pz, zi]
SLAB2_SHAPE = (128, NX2, 32, 2, NZS)
WA_COLS = 448    # [k0|k2 | k4|k6 | k1|k3 | k5]
W2_COLS = 2 * CO


# ---------------------------------------------------------------- host prep

def _assemble_kernel_sym(inp):
    """Assemble the dense conv kernel [64, 208, 7,7,7] and symmetrize the
    t-block -> [64, 160, 7,7,7]."""
    def blk(w, basis):
        w = np.asarray(w, np.float32)
        basis = np.asarray(basis, np.float32)
        mo, mi, nb = w.shape
        do, di = basis.shape[1], basis.shape[2]
        k = np.einsum('uvb,bijxyz->uivjxyz', w, basis)
        return k.reshape(mo * do, mi * di, SIZE, SIZE, SIZE)

    row_s = np.concatenate([blk(inp['w_ss'], inp['basis_ss']),
                            blk(inp['w_sv'], inp['basis_sv']),
                            blk(inp['w_st'], inp['basis_st'])], axis=1)
    row_v = np.concatenate([blk(inp['w_vs'], inp['basis_vs']),
                            blk(inp['w_vv'], inp['basis_vv']),
                            blk(inp['w_vt'], inp['basis_vt'])], axis=1)
    K = np.concatenate([row_s, row_v], axis=0)  # [64, 208, 7,7,7]

    Ks = np.empty((CO, CI, SIZE, SIZE, SIZE), np.float32)
    Ks[:, :64] = K[:, :64]
    for u in range(16):
        for pi, (i, j) in enumerate(PAIRS):
            src = K[:, 64 + 9 * u + 3 * i + j]
            if i != j:
                src = src + K[:, 64 + 9 * u + 3 * j + i]
            Ks[:, 64 + 6 * u + pi] = src
    return Ks


def _svt_sym(sv):
    """[4,64,32,32,32] -> symmetrized tensor-product features [4,160,32,32,32]."""
    sv = np.asarray(sv, np.float32)
    s = sv[:, :S_IN]
    v = sv[:, S_IN:].reshape(B, V_IN, 3, 32, 32, 32)
    t = np.empty((B, V_IN, 6, 32, 32, 32), np.float32)
    for pi, (i, j) in enumerate(PAIRS):
        t[:, :, pi] = v[:, :, i] * v[:, :, j]
    return np.concatenate([s, v.reshape(B, 48, 32, 32, 32),
                           t.reshape(B, 96, 32, 32, 32)], axis=1)


def _core_slabs(svt, b, h):
    """x/z zero-padded, z-parity-split slabs for core (b, h).

    c1 [128, 23, 32, 2, 19]: plane px holds global ix = px + 16h - 3.
    c2e [128, 10, 32, 2, 19]: block a (rows 32a:32a+32) of plane xi holds
    chunk-2 channels at ix = 2*xi + a + 16h - 3. zi_slab = zi_global + 2.
    """
    sp = svt[b].reshape(CI, 32, 32, 16, 2)   # (ci, x, y, zi, pz); iz = 2*zi + pz
    sp = np.moveaxis(sp, 4, 3)               # (ci, x, y, pz, zi)
    x0 = 16 * h - 3
    c1 = np.zeros(SLAB1_SHAPE, np.float32)
    lo, hi = max(0, x0), min(32, x0 + NXS)
    c1[:, lo - x0:hi - x0, :, :, 2:18] = sp[:128, lo:hi]
    c2e = np.zeros(SLAB2_SHAPE, np.float32)
    for a in range(4):
        for xi in range(NX2):
            ix = 2 * xi + a + x0
            if 0 <= ix < 32:
                c2e[32 * a:32 * a + 32, xi, :, :, 2:18] = sp[128:160, ix]
    return c1, c2e


def _weight_slabs(Ks):
    """(WA, W2). WA [49, 128, 448]: chunk-1 taps in column order
    [k0|k2|k4|k6|k1|k3|k5] (64 cols each); pairs (0,2),(4,6),(1,3) are the
    contiguous 128-col windows at 0, 128, 256; k5 singles at 384.
    W2 [49, 128, 128]: 4-way kx-merged chunk-2 (g=0: kx 0..3 lower 64 cols,
    g=1: kx 4..6 upper)."""
    KXORD = [0, 2, 4, 6, 1, 3, 5]
    WA = np.zeros((49, 128, WA_COLS), np.float32)
    W2 = np.zeros((49, 128, W2_COLS), np.float32)
    for ky, kz in product(range(SIZE), range(SIZE)):
        i = ky * SIZE + kz
        for ci, kx in enumerate(KXORD):
            WA[i, :, 64 * ci:64 * ci + 64] = Ks[:, :128, kx, ky, kz].T
        for g in range(2):
            for a in range(4):
                kx = 4 * g + a
                if kx > 6:
                    continue
                W2[i, 32 * a:32 * a + 32, 64 * g:64 * (g + 1)] = \
                    Ks[:, 128:160, kx, ky, kz].T
    return WA, W2


def _gam_bias(bn_g_s, bn_g_v, bias_s):
    """Per-channel gamma [64] (vector gammas replicated x3) and bias [64]."""
    gam = np.empty(64, np.float32)
    gam[:16] = np.asarray(bn_g_s, np.float32)
    gam[16:] = np.repeat(np.asarray(bn_g_v, np.float32), 3)
    bias = np.zeros(64, np.float32)
    bias[:16] = np.asarray(bias_s, np.float32)
    return gam, bias


# ---------------------------------------------------------------- matmul plan

def _box(ky, kz):
    """Valid output range + slab coords for kernel offsets (ky, kz)."""
    d = kz - 3
    p = d % 2
    zofs = (d - p) // 2
    oy0 = max(0, (4 - ky) // 2)
    oy1 = min(16, (34 - ky) // 2 + 1)
    iy0 = 2 * oy0 + ky - 3
    return dict(p=p, zs=zofs + 2, iy0=iy0, oyc=oy1 - oy0, oy0=oy0)


def _mm_plan():
    """Matmul descriptors (src, i, wc, ww, x0, bank) in issue order.

    Each matmul writes one full psum bank `bank` = slots (2*bank, 2*bank+1):
    partitions 0:ww x 2 halves x [oy0:oy0+oyc] x 16. Slot s holds plane s at
    partitions 0:64 (L) and plane s-1 at 64:128 (U). Chunk-2 c2p uses slots
    10..15 (banks 5-7): slot 10+j-2 L = plane j lower-taps, slot 10+j U =
    plane j upper-taps. Slot 8 L / slot 9 / U(0) are garbage sinks.

    Order: chunk-2 first (its half-size slab loads fast and its compute
    covers the big chunk-1 slab's DMA), then chunk-1.
    """
    plan = []
    for i in range(49):
        plan.append(('c2', i, 0, 64, 0, 0, 2))     # c2s g0 -> L(0),L(1)
        plan.append(('c2', i, 0, 128, 2, 5, 2))    # c2p -> slots (10,11)
        plan.append(('c2', i, 0, 128, 4, 6, 2))    # slots (12,13)
        plan.append(('c2', i, 0, 128, 6, 7, 2))    # slots (14,15)
        plan.append(('c2', i, 64, 64, 8, 3, 2))    # c2s g1 -> L(6),L(7)
    for i in range(49):
        for wc, ka in ((0, 0), (128, 4), (256, 1)):  # P(0,2), P(4,6), P(1,3)
            for bank in range(4):
                plan.append(('c1', i, wc, 128, ka + 4 * bank, bank, 2))
            # slot 9 is pure garbage, so the (s8,s9) pair shrinks to a
            # single-plane s8 matmul (only its U half, plane 7, is real)
            plan.append(('c1', i, wc, 128, ka + 16, 4, 1))
        for bank in range(4):                       # S5: px (5+4b), L only
            plan.append(('c1', i, 384, 64, 5 + 4 * bank, bank, 2))
    return plan


_PLAN = _mm_plan()

# stop flags: last matmul touching each psum bank
_LAST_IDX = {}
for _n, _d in enumerate(_PLAN):
    _LAST_IDX[_d[5]] = _n
_STOPS = set(_LAST_IDX.values())


# ---------------------------------------------------------------- numpy shadow

def _shadow_core(c1, c2e, WA, W2):
    """Execute the matmul plan in numpy. Returns conv output [64, 8, 16, 16]."""
    banks = np.zeros((8, 128, 2, 16, 16), np.float32)
    for src, i, wc, ww, x0, bank, npos in _PLAN:
        ky, kz = i // 7, i % 7
        bx = _box(ky, kz)
        sl = c2e if src == 'c2' else c1
        step = 1 if src == 'c2' else 2
        rhs = sl[:, x0:x0 + (npos - 1) * step + 1:step,
                 bx['iy0']:bx['iy0'] + 2 * bx['oyc']:2, bx['p'],
                 bx['zs']:bx['zs'] + 16]
        lhsT = (W2 if src == 'c2' else WA)[i][:, wc:wc + ww]
        contrib = np.einsum('km,kpbc->mpbc', lhsT, rhs)
        banks[bank][:ww, 0:npos, bx['oy0']:bx['oy0'] + bx['oyc'], :] += contrib
    out = np.empty((OXC, CO, 16, 16), np.float32)
    for j in range(OXC):
        acc = banks[j // 2][0:64, j % 2].copy()        # L(j)
        su = j + 1
        acc = acc + banks[su // 2][64:128, su % 2]     # U(j+1)
        if j >= 2:
            s = 8 + j                                  # LB: slot 10+j-2
            acc = acc + banks[s // 2][0:64, s % 2]
        if j <= 5:
            s = 10 + j                                 # UB: slot 10+j
            acc = acc + banks[s // 2][64:128, s % 2]
        out[j] = acc
    return out.transpose(1, 0, 2, 3)


def shadow_forward(inp):
    """Full-model numpy shadow of the device computation (for plan validation)."""
    svt = _svt_sym(inp['sv'])
    Ks = _assemble_kernel_sym(inp)
    WA, W2 = _weight_slabs(Ks)
    gam, bias = _gam_bias(inp['bn_g_s'], inp['bn_g_v'], inp['bias_s'])

    y = np.zeros((B, CO, 16, 16, 16), np.float32)
    ss = np.zeros(64, np.float64)
    for c in range(NCORES):
        b, h = c // 2, c % 2
        c1, c2e = _core_slabs(svt, b, h)
        out = _shadow_core(c1, c2e, WA, W2)
        y[b, :, 8 * h:8 * h + 8] = out
        ss += (out.astype(np.float64) ** 2).sum(axis=(1, 2, 3))

    var = np.empty(64)
    var[:16] = ss[:16] * VAR_S_DIV
    vv = (ss[16::3] + ss[17::3] + ss[18::3]) * VAR_V_DIV
    var[16:] = np.repeat(vv, 3)
    scale = gam / np.sqrt(var + EPS)
    y = y * scale[None, :, None, None, None].astype(np.float32)
    y[:, :16] = np.maximum(y[:, :16] + bias[:16][None, :, None, None, None], 0.0)
    return y


# ---------------------------------------------------------------- bass kernel

_CACHED = {}


def _build_bass():
    import concourse.bass as bass
    import concourse.tile as tile
    import concourse.mybir as mybir
    from concourse import bacc

    f32 = mybir.dt.float32
    f32r = mybir.dt.float32r
    bf16 = mybir.dt.bfloat16

    nc = bacc.Bacc("TRN2", target_bir_lowering=False, debug=False, num_devices=NCORES)

    in1 = nc.dram_tensor("in1", list(SLAB1_SHAPE), f32r, kind="ExternalInput").ap()
    in2e = nc.dram_tensor("in2e", list(SLAB2_SHAPE), f32r, kind="ExternalInput").ap()
    wa_in = nc.dram_tensor("wa_in", [49, 128, WA_COLS], f32r, kind="ExternalInput").ap()
    w2_in = nc.dram_tensor("w2_in", [128, 49 * W2_COLS], f32r, kind="ExternalInput").ap()
    gam_in = nc.dram_tensor("gam_in", [64, 1], f32, kind="ExternalInput").ap()
    bias_in = nc.dram_tensor("bias_in", [64, 1], f32, kind="ExternalInput").ap()
    out_d = nc.dram_tensor("out", [CO, OXC, 16, 16], f32, kind="ExternalOutput").ap()

    with tile.TileContext(nc) as tc:
        with (
            tc.tile_pool(name="slab", bufs=1) as slab_pool,
            tc.tile_pool(name="wp", bufs=4) as wpool,
            tc.tile_pool(name="ps", bufs=1, space="PSUM") as ps,
            tc.tile_pool(name="outp", bufs=1) as outp,
            tc.tile_pool(name="stat", bufs=1) as stat,
            tc.tile_pool(name="dram", bufs=1, space="DRAM") as dram,
        ):
            # 8 psum banks = 16 half-bank slots (garbage: 8L, 9, U(0))
            pq = [ps.tile([128, 2, 16, 16], f32, tag=f"pq{t}", name=f"pq{t}")
                  for t in range(8)]

            # tiny BN params first on the SP queue so they're resident early
            gam_t = stat.tile([CO, 1], f32, tag="gam")
            bias_t = stat.tile([CO, 1], f32, tag="bias")
            nc.sync.dma_start(gam_t[:], gam_in[:])
            nc.sync.dma_start(bias_t[:], bias_in[:])

            # slabs in bf16 (fp32r weights keep the contraction accurate; the
            # halved moving-operand footprint relieves SBUF read pressure and
            # halves slab DMA). Split across both HWDGE rings: SP carries the
            # early chunk-2 planes + chunk-1, Activation the late c2 planes.
            sl2 = slab_pool.tile(list(SLAB2_SHAPE), f32r, tag="slab2",
                                 name="slab_c2")
            sl1 = slab_pool.tile(list(SLAB1_SHAPE), f32r, tag="slab",
                                 name="slab_c1")
            for x in range(0, 10, 2):
                nc.sync.dma_start(sl2[:, x:x + 2], in2e[:, x:x + 2])
            nc.sync.dma_start(sl1[:, 0:11], in1[:, 0:11])
            nc.sync.dma_start(sl1[:, 11:NXS], in1[:, 11:NXS])

            # start=True clears the WHOLE psum bank, so open each bank once
            # with a zero-weight full-bank matmul (also a WAW dep that orders
            # it before every accumulate); all real matmuls use start=False.
            # rhs is a memset zeros tile so the opens run before any DMA lands.
            zw_f = stat.tile([128, 128], f32, tag="zw")
            zr_f = stat.tile([128, 512], f32, tag="zr")
            nc.vector.memset(zw_f[:], 0.0)
            nc.vector.memset(zr_f[:], 0.0)
            zw = zw_f.bitcast(f32r)
            zr = zr_f.bitcast(f32r)
            for t in range(8):
                nc.tensor.matmul(pq[t].rearrange("c a y z -> c (a y z)"),
                                 zw[:], zr[:], start=True, stop=False)

            # all chunk-2 weights live in ONE resident tile: a [128 x 25KB]
            # DMA is 128 large descriptors (descriptor-efficient), so chunk-2
            # never starves; chunk-1 tiles trickle on the slow Activation
            # ring, which keeps bulk SBUF writes from contending with the
            # fast-mode matmul stream
            w2all = wpool.tile([128, 49 * W2_COLS], f32r, tag="w2a",
                               name="w2all", bufs=1)
            nc.scalar.dma_start(w2all[:], w2_in[:])
            wat = {}
            for i in range(49):
                w = wpool.tile([128, WA_COLS], f32r, tag="wa", name=f"wa_{i}",
                               bufs=4)
                nc.scalar.dma_start(w[:], wa_in[i])
                wat[i] = w

            # warm up the collective engine so the tail AllReduce skips the
            # ~11us first-use wake latency (result discarded)
            cc_w_in = dram.tile([1, 64], f32, tag="ccwi")
            cc_w_out = dram.tile([1, 64], f32, tag="ccwo")
            nc.gpsimd.collective_compute(
                "AllReduce", mybir.AluOpType.add,
                replica_groups=[list(range(NCORES))],
                ins=[cc_w_in.opt()], outs=[cc_w_out.opt()],
            )

            for n, (src, i, wc, ww, x0, bank, npos) in enumerate(_PLAN):
                ky, kz = i // 7, i % 7
                bx = _box(ky, kz)
                if src == 'c2':
                    lhsT = w2all[:, i * W2_COLS + wc:i * W2_COLS + wc + ww]
                    sl, step = sl2, 1
                else:
                    lhsT = wat[i][:, wc:wc + ww]
                    sl, step = sl1, 2
                yslice = slice(bx['iy0'], bx['iy0'] + 2 * bx['oyc'] - 1, 2)
                if npos == 2:
                    rhs = sl[:, x0:x0 + step + 1:step, yslice, bx['p'],
                             bx['zs']:bx['zs'] + 16]
                    out_ap = pq[bank][0:ww, 0:2,
                                      bx['oy0']:bx['oy0'] + bx['oyc'], :]
                else:
                    rhs = sl[:, x0, yslice, bx['p'], bx['zs']:bx['zs'] + 16]
                    out_ap = pq[bank][0:ww, 0,
                                      bx['oy0']:bx['oy0'] + bx['oyc'], :]
                nc.tensor.matmul(out_ap, lhsT, rhs,
                                 start=False, stop=n in _STOPS)

            # evacuate: plane j = L(j) + U(j+1) [+ LB(10+j-2) j>=2]
            #                                   [+ UB(10+j)   j<=5]
            # a TensorTensor may read only ONE psum operand: Scalar engine
            # copies psum->sbuf, Vector accumulates the second psum operand
            osb = outp.tile([CO, OXC, 16, 16], f32, tag="osb")
            usb = outp.tile([128, OXC, 16, 16], f32, tag="usb")
            for j in range(OXC):
                nc.scalar.activation(osb[:, j], pq[j // 2][0:64, j % 2],
                                     mybir.ActivationFunctionType.Copy,
                                     scale=1.0)
                if j >= 2:
                    s = 8 + j
                    nc.vector.tensor_add(osb[:, j], osb[:, j],
                                         pq[s // 2][0:64, s % 2])
                su = j + 1
                nc.scalar.activation(usb[64:128, j],
                                     pq[su // 2][64:128, su % 2],
                                     mybir.ActivationFunctionType.Copy,
                                     scale=1.0)
                if j <= 5:
                    s = 10 + j
                    nc.vector.tensor_add(usb[64:128, j], usb[64:128, j],
                                         pq[s // 2][64:128, s % 2])
            # rendezvous: a 4-byte token dependent on the first evacuated
            # plane enters a dummy AllReduce, aligning the cores while the
            # evac/stats chain runs, so the real AllReduce's peer wait
            # overlaps work instead of extending the tail
            cc_b_in = dram.tile([1, 64], f32, tag="ccbi")
            cc_b_out = dram.tile([1, 64], f32, tag="ccbo")
            nc.sync.dma_start(cc_b_in[0:1, 0:1], osb[0:1, 0, 0, 0:1])
            nc.gpsimd.collective_compute(
                "AllReduce", mybir.AluOpType.add,
                replica_groups=[list(range(NCORES))],
                ins=[cc_b_in.opt()], outs=[cc_b_out.opt()],
            )

            # move upper-half partials down to partitions 0:64 and add
            u_dram = dram.tile([64, OXC, 16, 16], f32, tag="ud")
            nc.sync.dma_start(u_dram[:], usb[64:128])
            nc.sync.dma_start(usb[0:64], u_dram[:])
            of = osb.rearrange("c x y z -> c (x y z)")
            uf = usb.rearrange("c x y z -> c (x y z)")
            nc.vector.tensor_add(of[:, :], of[:, :], uf[0:64, :])

            # per-channel sum of squares in ONE scalar-engine op (Square with
            # free-axis accumulator) -> local variance contribution
            # (linear in the sums, so the AllReduce can carry variance
            # directly and the post-collective chain stays short)
            sq = outp.tile([CO, 2048], f32, tag="sq")
            ssq = stat.tile([CO, 1], f32, tag="ssq")
            nc.scalar.activation(sq[:], of[:, :],
                                 mybir.ActivationFunctionType.Square,
                                 scale=1.0, accum_out=ssq[:, :])
            ss_row = stat.tile([1, 64], f32, tag="ssrow")
            vloc = stat.tile([1, 64], f32, tag="vloc")
            tmp16 = stat.tile([1, 16], f32, tag="tmp16")
            ss_dram = dram.tile([1, 64], f32, tag="ssd")
            nc.sync.dma_start(ss_dram[0, :], ssq[:, 0])
            nc.sync.dma_start(ss_row[:], ss_dram[:])
            nc.vector.tensor_add(tmp16[:], ss_row[:, 16::3], ss_row[:, 17::3])
            nc.vector.tensor_add(tmp16[:], tmp16[:], ss_row[:, 18::3])
            nc.vector.tensor_scalar_mul(vloc[:, 0:16], ss_row[:, 0:16], VAR_S_DIV)
            for j in range(3):
                nc.vector.tensor_scalar_mul(vloc[:, 16 + j::3], tmp16[:], VAR_V_DIV)

            v_dram = dram.tile([1, 64], f32, tag="vd")
            v_red = dram.tile([1, 64], f32, tag="vr")
            nc.sync.dma_start(v_dram[:], vloc[:])
            nc.gpsimd.collective_compute(
                "AllReduce", mybir.AluOpType.add,
                replica_groups=[list(range(NCORES))],
                ins=[v_dram.opt()], outs=[v_red.opt()],
            )

            # scale = gamma / sqrt(var + eps), in per-partition layout
            var_col = stat.tile([CO, 1], f32, tag="varcol")
            nc.sync.dma_start(var_col[:, 0], v_red[0, :])
            eps_t = stat.tile([CO, 1], f32, tag="eps")
            nc.vector.memset(eps_t[:], EPS)
            sd = stat.tile([CO, 1], f32, tag="sd")
            nc.scalar.activation(sd[:], var_col[:], mybir.ActivationFunctionType.Sqrt,
                                 bias=eps_t[:], scale=1.0)
            inv = stat.tile([CO, 1], f32, tag="inv")
            nc.vector.reciprocal(inv[:], sd[:])
            scale_col = stat.tile([CO, 1], f32, tag="sccol")
            nc.vector.tensor_mul(scale_col[:], inv[:], gam_t[:])

            # apply BN scale everywhere, then bias+relu on scalar channels
            nc.vector.tensor_scalar_mul(of[:, :], of[:, :], scale_col[:, :])
            nc.scalar.activation(of[0:16, :], of[0:16, :],
                                 mybir.ActivationFunctionType.Relu,
                                 bias=bias_t[0:16, :], scale=1.0)
            nc.sync.dma_start(out_d[:], osb[:])

    nc.compile()
    return nc


def _install_ntff_hook():
    import sys, types
    if "antenv.axon_hooks" in sys.modules:
        return
    mod = types.ModuleType("antenv.axon_hooks")
    mod._hook = None
    mod.set_axon_ntff_profile_hook = lambda h: setattr(mod, "_hook", h)
    mod.get_axon_ntff_profile_hook = lambda: mod._hook
    sys.modules["antenv.axon_hooks"] = mod
    try:
        import antenv
        antenv.axon_hooks = mod
        from trn_agent_boot.trn_boot import _ntff_profile_via_ctypes
        mod.set_axon_ntff_profile_hook(_ntff_profile_via_ctypes("/opt/axon/libaxon_pjrt.so"))
    except Exception:
        pass


def run_on_hw(inp, trace=False):
    """Run the kernel on 8 cores. Returns (full output [4,64,16,16,16], results)."""
    from concourse.bass_utils import run_bass_kernel_spmd

    if "nc" not in _CACHED:
        _install_ntff_hook()
        _CACHED["nc"] = _build_bass()
    nc = _CACHED["nc"]

    svt = _svt_sym(inp['sv'])
    Ks = _assemble_kernel_sym(inp)
    WA, W2 = _weight_slabs(Ks)
    gam, bias = _gam_bias(inp['bn_g_s'], inp['bn_g_v'], inp['bias_s'])

    in_maps = []
    for c in range(NCORES):
        b, h = c // 2, c % 2
        c1, c2e = _core_slabs(svt, b, h)
        in_maps.append({
            "in1": c1,
            "in2e": c2e,
            "wa_in": WA,
            "w2_in": np.ascontiguousarray(W2.transpose(1, 0, 2).reshape(128, 49 * W2_COLS)),
            "gam_in": gam.reshape(64, 1),
            "bias_in": bias.reshape(64, 1),
        })

    res = run_bass_kernel_spmd(nc, in_maps, core_ids=list(range(NCORES)), trace=trace)

    y = np.zeros((B, CO, 16, 16, 16), np.float32)
    for c in range(NCORES):
        b, h = c // 2, c % 2
        y[b, :, 8 * h:8 * h + 8] = res.results[c]["out"]
    return y, res


def kernel(**inputs) -> np.ndarray:
    y, _ = run_on_hw(inputs, trace=False)
    return y


# revision 17
# speedup vs baseline: 1.3224x; 1.0016x over previous
"""SE(3)-CNN block (TensorProduct -> SE3Conv -> SE3BatchNorm -> BiasRelu) on 8 trn2 cores.

Sharding: core c = (batch b=c//2, out-x-half h=c%2). Each core computes all 64
output channels for 8 of 16 output x-planes of one batch; per-field BN second
moments are combined with a tiny [1,64] AllReduce across all 8 cores.

v2 conv strategy (vs single-plane baseline):
- Every fp32r matmul covers TWO output x-planes = one full psum bank, free
  dim 448-512 (>= 256 keeps fp32r in its fast streaming mode) and half the
  instruction count.
- No M=64 boundary singles: each chunk-1 kx pair block runs its full slot
  range (s=0..9); out-of-range tap contributions land in psum half-banks the
  evacuation never reads (slot 8 L, slot 9) fed from x-padded slab planes.
- chunk2 (channels 128:160, 4 kx taps packed per 128-row contraction) uses
  slots 10-15 (banks 5-7) plus L(0),L(1),L(6),L(7).
- Weight slab dedup: chunk-1 kx blocks stored once in column order
  [0,2 | 4,6 | 1,3 | 5] so every used pair is contiguous (11.2MB vs 24MB).
- Weight DMA on the Activation HWDGE queue, slabs + tail/stat DMAs on SP,
  evacuation adds split across Vector and GpSimd.
"""
import numpy as np
from itertools import product

# problem constants (from spec / reference)
B = 4
S_IN = 16
V_IN = 16
CO = 64          # 16 scalar + 48 vector output channels
CI = 160         # 16 s + 48 v + 96 t_sym
SIZE = 7
PAD = 3
STRIDE = 2
EPS = 1e-5
NCORES = 8
NXS = 21         # x-padded slab planes per core
NZS = 19         # z-padded: zi_slab = zi_global + 2, covering zofs in [-2, 1]
NX2 = 10         # chunk-2 even slab planes (px = 2*xi, xi 0..9)
OXC = 8          # out x-planes per core
PAIRS = [(0, 0), (0, 1), (0, 2), (1, 1), (1, 2), (2, 2)]
VAR_S_DIV = 1.0 / (B * 16 * 16 * 16)
VAR_V_DIV = 1.0 / (B * 3 * 16 * 16 * 16)

SLAB1_SHAPE = (128, NXS, 32, 2, NZS)   # [ci, px, iy, pz, zi]
SLAB2_SHAPE = (128, NX2, 32, 2, NZS)
WA_COLS = 448    # [k0|k2 | k4|k6 | k1|k3 | k5]
W2_COLS = 2 * CO


# ---------------------------------------------------------------- host prep

def _assemble_kernel_sym(inp):
    """Assemble the dense conv kernel [64, 208, 7,7,7] and symmetrize the
    t-block -> [64, 160, 7,7,7]."""
    def blk(w, basis):
        w = np.asarray(w, np.float32)
        basis = np.asarray(basis, np.float32)
        mo, mi, nb = w.shape
        do, di = basis.shape[1], basis.shape[2]
        k = np.einsum('uvb,bijxyz->uivjxyz', w, basis)
        return k.reshape(mo * do, mi * di, SIZE, SIZE, SIZE)

    row_s = np.concatenate([blk(inp['w_ss'], inp['basis_ss']),
                            blk(inp['w_sv'], inp['basis_sv']),
                            blk(inp['w_st'], inp['basis_st'])], axis=1)
    row_v = np.concatenate([blk(inp['w_vs'], inp['basis_vs']),
                            blk(inp['w_vv'], inp['basis_vv']),
                            blk(inp['w_vt'], inp['basis_vt'])], axis=1)
    K = np.concatenate([row_s, row_v], axis=0)  # [64, 208, 7,7,7]

    Ks = np.empty((CO, CI, SIZE, SIZE, SIZE), np.float32)
    Ks[:, :64] = K[:, :64]
    for u in range(16):
        for pi, (i, j) in enumerate(PAIRS):
            src = K[:, 64 + 9 * u + 3 * i + j]
            if i != j:
                src = src + K[:, 64 + 9 * u + 3 * j + i]
            Ks[:, 64 + 6 * u + pi] = src
    return Ks


def _svt_sym(sv):
    """[4,64,32,32,32] -> symmetrized tensor-product features [4,160,32,32,32]."""
    sv = np.asarray(sv, np.float32)
    s = sv[:, :S_IN]
    v = sv[:, S_IN:].reshape(B, V_IN, 3, 32, 32, 32)
    t = np.empty((B, V_IN, 6, 32, 32, 32), np.float32)
    for pi, (i, j) in enumerate(PAIRS):
        t[:, :, pi] = v[:, :, i] * v[:, :, j]
    return np.concatenate([s, v.reshape(B, 48, 32, 32, 32),
                           t.reshape(B, 96, 32, 32, 32)], axis=1)


def _core_slabs(svt, b, h):
    """x/z zero-padded, z-parity-split slabs for core (b, h).

    c1 [128, 23, 32, 2, 19]: plane px holds global ix = px + 16h - 3.
    c2e [128, 10, 32, 2, 19]: block a (rows 32a:32a+32) of plane xi holds
    chunk-2 channels at ix = 2*xi + a + 16h - 3. zi_slab = zi_global + 2.
    """
    sp = svt[b].reshape(CI, 32, 32, 16, 2)   # (ci, x, y, zi, pz); iz = 2*zi + pz
    sp = np.moveaxis(sp, 4, 3)               # (ci, x, y, pz, zi)
    x0 = 16 * h - 3
    c1 = np.zeros(SLAB1_SHAPE, np.float32)
    lo, hi = max(0, x0), min(32, x0 + NXS)
    c1[:, lo - x0:hi - x0, :, :, 2:18] = sp[:128, lo:hi]
    c2e = np.zeros(SLAB2_SHAPE, np.float32)
    for a in range(4):
        for xi in range(NX2):
            ix = 2 * xi + a + x0
            if 0 <= ix < 32:
                c2e[32 * a:32 * a + 32, xi, :, :, 2:18] = sp[128:160, ix]
    return c1, c2e


def _weight_slabs(Ks):
    """(WA, W2). WA [49, 128, 448]: chunk-1 taps in column order
    [k0|k2|k4|k6|k1|k3|k5] (64 cols each); pairs (0,2),(4,6),(1,3) are the
    contiguous 128-col windows at 0, 128, 256; k5 singles at 384.
    W2 [49, 128, 128]: 4-way kx-merged chunk-2 (g=0: kx 0..3 lower 64 cols,
    g=1: kx 4..6 upper)."""
    KXORD = [0, 2, 4, 6, 1, 3, 5]
    WA = np.zeros((49, 128, WA_COLS), np.float32)
    W2 = np.zeros((49, 128, W2_COLS), np.float32)
    for ky, kz in product(range(SIZE), range(SIZE)):
        i = ky * SIZE + kz
        for ci, kx in enumerate(KXORD):
            WA[i, :, 64 * ci:64 * ci + 64] = Ks[:, :128, kx, ky, kz].T
        for g in range(2):
            for a in range(4):
                kx = 4 * g + a
                if kx > 6:
                    continue
                W2[i, 32 * a:32 * a + 32, 64 * g:64 * (g + 1)] = \
                    Ks[:, 128:160, kx, ky, kz].T
    return WA, W2


def _gam_bias(bn_g_s, bn_g_v, bias_s):
    """Per-channel gamma [64] (vector gammas replicated x3) and bias [64]."""
    gam = np.empty(64, np.float32)
    gam[:16] = np.asarray(bn_g_s, np.float32)
    gam[16:] = np.repeat(np.asarray(bn_g_v, np.float32), 3)
    bias = np.zeros(64, np.float32)
    bias[:16] = np.asarray(bias_s, np.float32)
    return gam, bias


# ---------------------------------------------------------------- matmul plan

def _box(ky, kz):
    """Valid output range + slab coords for kernel offsets (ky, kz)."""
    d = kz - 3
    p = d % 2
    zofs = (d - p) // 2
    oy0 = max(0, (4 - ky) // 2)
    oy1 = min(16, (34 - ky) // 2 + 1)
    iy0 = 2 * oy0 + ky - 3
    return dict(p=p, zs=zofs + 2, iy0=iy0, oyc=oy1 - oy0, oy0=oy0)


def _mm_plan():
    """Matmul descriptors (src, i, wc, ww, x0, bank) in issue order.

    Each matmul writes one full psum bank `bank` = slots (2*bank, 2*bank+1):
    partitions 0:ww x 2 halves x [oy0:oy0+oyc] x 16. Slot s holds plane s at
    partitions 0:64 (L) and plane s-1 at 64:128 (U). Chunk-2 c2p uses slots
    10..15 (banks 5-7): slot 10+j-2 L = plane j lower-taps, slot 10+j U =
    plane j upper-taps. Slot 8 L / slot 9 / U(0) are garbage sinks.

    Order: chunk-2 first (its half-size slab loads fast and its compute
    covers the big chunk-1 slab's DMA), then chunk-1.
    """
    plan = []
    for i in range(49):
        plan.append(('c2', i, 0, 64, 0, 0, 2))     # c2s g0 -> L(0),L(1)
        plan.append(('c2', i, 0, 128, 2, 5, 2))    # c2p -> slots (10,11)
        plan.append(('c2', i, 0, 128, 4, 6, 2))    # slots (12,13)
        plan.append(('c2', i, 0, 128, 6, 7, 2))    # slots (14,15)
        plan.append(('c2', i, 64, 64, 8, 3, 2))    # c2s g1 -> L(6),L(7)
    for i in range(49):
        for wc, ka in ((0, 0), (128, 4), (256, 1)):  # P(0,2), P(4,6), P(1,3)
            for bank in range(4):
                plan.append(('c1', i, wc, 128, ka + 4 * bank, bank, 2))
            # slot 9 is pure garbage, so the (s8,s9) pair shrinks to a
            # single-plane s8 matmul (only its U half, plane 7, is real)
            plan.append(('c1', i, wc, 128, ka + 16, 4, 1))
        for bank in range(4):                       # S5: px (5+4b), L only
            plan.append(('c1', i, 384, 64, 5 + 4 * bank, bank, 2))
    return plan


_PLAN = _mm_plan()

# stop flags: last matmul touching each psum bank
_LAST_IDX = {}
for _n, _d in enumerate(_PLAN):
    _LAST_IDX[_d[5]] = _n
_STOPS = set(_LAST_IDX.values())


# ---------------------------------------------------------------- numpy shadow

def _shadow_core(c1, c2e, WA, W2):
    """Execute the matmul plan in numpy. Returns conv output [64, 8, 16, 16]."""
    banks = np.zeros((8, 128, 2, 16, 16), np.float32)
    for src, i, wc, ww, x0, bank, npos in _PLAN:
        ky, kz = i // 7, i % 7
        bx = _box(ky, kz)
        sl = c2e if src == 'c2' else c1
        step = 1 if src == 'c2' else 2
        rhs = sl[:, x0:x0 + (npos - 1) * step + 1:step,
                 bx['iy0']:bx['iy0'] + 2 * bx['oyc']:2, bx['p'],
                 bx['zs']:bx['zs'] + 16]
        lhsT = (W2 if src == 'c2' else WA)[i][:, wc:wc + ww]
        contrib = np.einsum('km,kpbc->mpbc', lhsT, rhs)
        banks[bank][:ww, 0:npos, bx['oy0']:bx['oy0'] + bx['oyc'], :] += contrib
    out = np.empty((OXC, CO, 16, 16), np.float32)
    for j in range(OXC):
        acc = banks[j // 2][0:64, j % 2].copy()        # L(j)
        su = j + 1
        acc = acc + banks[su // 2][64:128, su % 2]     # U(j+1)
        if j >= 2:
            s = 8 + j                                  # LB: slot 10+j-2
            acc = acc + banks[s // 2][0:64, s % 2]
        if j <= 5:
            s = 10 + j                                 # UB: slot 10+j
            acc = acc + banks[s // 2][64:128, s % 2]
        out[j] = acc
    return out.transpose(1, 0, 2, 3)


def shadow_forward(inp):
    """Full-model numpy shadow of the device computation (for plan validation)."""
    svt = _svt_sym(inp['sv'])
    Ks = _assemble_kernel_sym(inp)
    WA, W2 = _weight_slabs(Ks)
    gam, bias = _gam_bias(inp['bn_g_s'], inp['bn_g_v'], inp['bias_s'])

    y = np.zeros((B, CO, 16, 16, 16), np.float32)
    ss = np.zeros(64, np.float64)
    for c in range(NCORES):
        b, h = c // 2, c % 2
        c1, c2e = _core_slabs(svt, b, h)
        out = _shadow_core(c1, c2e, WA, W2)
        y[b, :, 8 * h:8 * h + 8] = out
        ss += (out.astype(np.float64) ** 2).sum(axis=(1, 2, 3))

    var = np.empty(64)
    var[:16] = ss[:16] * VAR_S_DIV
    vv = (ss[16::3] + ss[17::3] + ss[18::3]) * VAR_V_DIV
    var[16:] = np.repeat(vv, 3)
    scale = gam / np.sqrt(var + EPS)
    y = y * scale[None, :, None, None, None].astype(np.float32)
    y[:, :16] = np.maximum(y[:, :16] + bias[:16][None, :, None, None, None], 0.0)
    return y


# ---------------------------------------------------------------- bass kernel

_CACHED = {}


def _build_bass():
    import concourse.bass as bass
    import concourse.tile as tile
    import concourse.mybir as mybir
    from concourse import bacc

    f32 = mybir.dt.float32
    f32r = mybir.dt.float32r
    bf16 = mybir.dt.bfloat16

    nc = bacc.Bacc("TRN2", target_bir_lowering=False, debug=False, num_devices=NCORES)

    in1 = nc.dram_tensor("in1", list(SLAB1_SHAPE), f32r, kind="ExternalInput").ap()
    in2e = nc.dram_tensor("in2e", list(SLAB2_SHAPE), f32r, kind="ExternalInput").ap()
    wa_in = nc.dram_tensor("wa_in", [49, 128, WA_COLS], f32r, kind="ExternalInput").ap()
    w2_in = nc.dram_tensor("w2_in", [128, 49 * W2_COLS], f32r, kind="ExternalInput").ap()
    gam_in = nc.dram_tensor("gam_in", [64, 1], f32, kind="ExternalInput").ap()
    bias_in = nc.dram_tensor("bias_in", [64, 1], f32, kind="ExternalInput").ap()
    out_d = nc.dram_tensor("out", [CO, OXC, 16, 16], f32, kind="ExternalOutput").ap()

    with tile.TileContext(nc) as tc:
        with (
            tc.tile_pool(name="slab", bufs=1) as slab_pool,
            tc.tile_pool(name="wp", bufs=4) as wpool,
            tc.tile_pool(name="ps", bufs=1, space="PSUM") as ps,
            tc.tile_pool(name="outp", bufs=1) as outp,
            tc.tile_pool(name="stat", bufs=1) as stat,
            tc.tile_pool(name="dram", bufs=1, space="DRAM") as dram,
        ):
            # 8 psum banks = 16 half-bank slots (garbage: 8L, 9, U(0))
            pq = [ps.tile([128, 2, 16, 16], f32, tag=f"pq{t}", name=f"pq{t}")
                  for t in range(8)]

            # tiny BN params first on the SP queue so they're resident early
            gam_t = stat.tile([CO, 1], f32, tag="gam")
            bias_t = stat.tile([CO, 1], f32, tag="bias")
            nc.sync.dma_start(gam_t[:], gam_in[:])
            nc.sync.dma_start(bias_t[:], bias_in[:])

            # slabs in bf16 (fp32r weights keep the contraction accurate; the
            # halved moving-operand footprint relieves SBUF read pressure and
            # halves slab DMA). Split across both HWDGE rings: SP carries the
            # early chunk-2 planes + chunk-1, Activation the late c2 planes.
            sl2 = slab_pool.tile(list(SLAB2_SHAPE), f32r, tag="slab2",
                                 name="slab_c2")
            sl1 = slab_pool.tile(list(SLAB1_SHAPE), f32r, tag="slab",
                                 name="slab_c1")
            for x in range(0, 10, 2):
                nc.sync.dma_start(sl2[:, x:x + 2], in2e[:, x:x + 2])
            nc.sync.dma_start(sl1[:, 0:11], in1[:, 0:11])
            nc.sync.dma_start(sl1[:, 11:NXS], in1[:, 11:NXS])

            # start=True clears the WHOLE psum bank, so open each bank once
            # with a zero-weight full-bank matmul (also a WAW dep that orders
            # it before every accumulate); all real matmuls use start=False.
            # rhs is a memset zeros tile so the opens run before any DMA lands.
            zw_f = stat.tile([128, 128], f32, tag="zw")
            zr_f = stat.tile([128, 512], f32, tag="zr")
            nc.vector.memset(zw_f[:], 0.0)
            nc.vector.memset(zr_f[:], 0.0)
            zw = zw_f.bitcast(f32r)
            zr = zr_f.bitcast(f32r)
            for t in range(8):
                nc.tensor.matmul(pq[t].rearrange("c a y z -> c (a y z)"),
                                 zw[:], zr[:], start=True, stop=False)

            # all chunk-2 weights live in ONE resident tile: a [128 x 25KB]
            # DMA is 128 large descriptors (descriptor-efficient), so chunk-2
            # never starves; chunk-1 tiles trickle on the slow Activation
            # ring, which keeps bulk SBUF writes from contending with the
            # fast-mode matmul stream
            w2all = wpool.tile([128, 49 * W2_COLS], f32r, tag="w2a",
                               name="w2all", bufs=1)
            nc.scalar.dma_start(w2all[:, :24 * W2_COLS], w2_in[:, :24 * W2_COLS])
            nc.scalar.dma_start(w2all[:, 24 * W2_COLS:], w2_in[:, 24 * W2_COLS:])
            wat = {}
            for i in range(49):
                w = wpool.tile([128, WA_COLS], f32r, tag="wa", name=f"wa_{i}",
                               bufs=4)
                nc.scalar.dma_start(w[:], wa_in[i])
                wat[i] = w


            for n, (src, i, wc, ww, x0, bank, npos) in enumerate(_PLAN):
                ky, kz = i // 7, i % 7
                bx = _box(ky, kz)
                if src == 'c2':
                    lhsT = w2all[:, i * W2_COLS + wc:i * W2_COLS + wc + ww]
                    sl, step = sl2, 1
                else:
                    lhsT = wat[i][:, wc:wc + ww]
                    sl, step = sl1, 2
                yslice = slice(bx['iy0'], bx['iy0'] + 2 * bx['oyc'] - 1, 2)
                if npos == 2:
                    rhs = sl[:, x0:x0 + step + 1:step, yslice, bx['p'],
                             bx['zs']:bx['zs'] + 16]
                    out_ap = pq[bank][0:ww, 0:2,
                                      bx['oy0']:bx['oy0'] + bx['oyc'], :]
                else:
                    rhs = sl[:, x0, yslice, bx['p'], bx['zs']:bx['zs'] + 16]
                    out_ap = pq[bank][0:ww, 0,
                                      bx['oy0']:bx['oy0'] + bx['oyc'], :]
                nc.tensor.matmul(out_ap, lhsT, rhs,
                                 start=False, stop=n in _STOPS)

            # evacuate: plane j = L(j) + U(j+1) [+ LB(10+j-2) j>=2]
            #                                   [+ UB(10+j)   j<=5]
            # a TensorTensor may read only ONE psum operand: Scalar engine
            # copies psum->sbuf, Vector accumulates the second psum operand
            osb = outp.tile([CO, OXC, 16, 16], f32, tag="osb")
            usb = outp.tile([128, OXC, 16, 16], f32, tag="usb")
            for j in range(OXC):
                nc.scalar.activation(osb[:, j], pq[j // 2][0:64, j % 2],
                                     mybir.ActivationFunctionType.Copy,
                                     scale=1.0)
                if j >= 2:
                    s = 8 + j
                    nc.vector.tensor_add(osb[:, j], osb[:, j],
                                         pq[s // 2][0:64, s % 2])
                su = j + 1
                nc.scalar.activation(usb[64:128, j],
                                     pq[su // 2][64:128, su % 2],
                                     mybir.ActivationFunctionType.Copy,
                                     scale=1.0)
                if j <= 5:
                    s = 10 + j
                    nc.vector.tensor_add(usb[64:128, j], usb[64:128, j],
                                         pq[s // 2][64:128, s % 2])
            # rendezvous: a 4-byte token dependent on the first evacuated
            # plane enters a dummy AllReduce, aligning the cores while the
            # evac/stats chain runs, so the real AllReduce's peer wait
            # overlaps work instead of extending the tail
            cc_b_in = dram.tile([1, 64], f32, tag="ccbi")
            cc_b_out = dram.tile([1, 64], f32, tag="ccbo")
            nc.sync.dma_start(cc_b_in[0:1, 0:1], osb[0:1, 0, 0, 0:1])
            nc.gpsimd.collective_compute(
                "AllReduce", mybir.AluOpType.add,
                replica_groups=[list(range(NCORES))],
                ins=[cc_b_in.opt()], outs=[cc_b_out.opt()],
            )

            # move upper-half partials down to partitions 0:64 and add
            u_dram = dram.tile([64, OXC, 16, 16], f32, tag="ud")
            nc.sync.dma_start(u_dram[:], usb[64:128])
            nc.sync.dma_start(usb[0:64], u_dram[:])
            of = osb.rearrange("c x y z -> c (x y z)")
            uf = usb.rearrange("c x y z -> c (x y z)")
            nc.vector.tensor_add(of[:, :], of[:, :], uf[0:64, :])

            # per-channel sum of squares in ONE scalar-engine op (Square with
            # free-axis accumulator) -> local variance contribution
            # (linear in the sums, so the AllReduce can carry variance
            # directly and the post-collective chain stays short)
            sq = outp.tile([CO, 2048], f32, tag="sq")
            ssq = stat.tile([CO, 1], f32, tag="ssq")
            nc.scalar.activation(sq[:], of[:, :],
                                 mybir.ActivationFunctionType.Square,
                                 scale=1.0, accum_out=ssq[:, :])
            ss_row = stat.tile([1, 64], f32, tag="ssrow")
            vloc = stat.tile([1, 64], f32, tag="vloc")
            tmp16 = stat.tile([1, 16], f32, tag="tmp16")
            ss_dram = dram.tile([1, 64], f32, tag="ssd")
            nc.sync.dma_start(ss_dram[0, :], ssq[:, 0])
            nc.sync.dma_start(ss_row[:], ss_dram[:])
            nc.vector.tensor_add(tmp16[:], ss_row[:, 16::3], ss_row[:, 17::3])
            nc.vector.tensor_add(tmp16[:], tmp16[:], ss_row[:, 18::3])
            nc.vector.tensor_scalar_mul(vloc[:, 0:16], ss_row[:, 0:16], VAR_S_DIV)
            for j in range(3):
                nc.vector.tensor_scalar_mul(vloc[:, 16 + j::3], tmp16[:], VAR_V_DIV)

            v_dram = dram.tile([1, 64], f32, tag="vd")
            v_red = dram.tile([1, 64], f32, tag="vr")
            nc.sync.dma_start(v_dram[:], vloc[:])
            nc.gpsimd.collective_compute(
                "AllReduce", mybir.AluOpType.add,
                replica_groups=[list(range(NCORES))],
                ins=[v_dram.opt()], outs=[v_red.opt()],
            )

            # scale = gamma / sqrt(var + eps), in per-partition layout
            var_col = stat.tile([CO, 1], f32, tag="varcol")
            nc.sync.dma_start(var_col[:, 0], v_red[0, :])
            eps_t = stat.tile([CO, 1], f32, tag="eps")
            nc.vector.memset(eps_t[:], EPS)
            sd = stat.tile([CO, 1], f32, tag="sd")
            nc.scalar.activation(sd[:], var_col[:], mybir.ActivationFunctionType.Sqrt,
                                 bias=eps_t[:], scale=1.0)
            inv = stat.tile([CO, 1], f32, tag="inv")
            nc.vector.reciprocal(inv[:], sd[:])
            scale_col = stat.tile([CO, 1], f32, tag="sccol")
            nc.vector.tensor_mul(scale_col[:], inv[:], gam_t[:])

            # apply BN scale everywhere, then bias+relu on scalar channels
            nc.vector.tensor_scalar_mul(of[:, :], of[:, :], scale_col[:, :])
            nc.scalar.activation(of[0:16, :], of[0:16, :],
                                 mybir.ActivationFunctionType.Relu,
                                 bias=bias_t[0:16, :], scale=1.0)
            nc.sync.dma_start(out_d[:], osb[:])

    nc.compile()
    return nc


def _install_ntff_hook():
    import sys, types
    if "antenv.axon_hooks" in sys.modules:
        return
    mod = types.ModuleType("antenv.axon_hooks")
    mod._hook = None
    mod.set_axon_ntff_profile_hook = lambda h: setattr(mod, "_hook", h)
    mod.get_axon_ntff_profile_hook = lambda: mod._hook
    sys.modules["antenv.axon_hooks"] = mod
    try:
        import antenv
        antenv.axon_hooks = mod
        from trn_agent_boot.trn_boot import _ntff_profile_via_ctypes
        mod.set_axon_ntff_profile_hook(_ntff_profile_via_ctypes("/opt/axon/libaxon_pjrt.so"))
    except Exception:
        pass


def run_on_hw(inp, trace=False):
    """Run the kernel on 8 cores. Returns (full output [4,64,16,16,16], results)."""
    from concourse.bass_utils import run_bass_kernel_spmd

    if "nc" not in _CACHED:
        _install_ntff_hook()
        _CACHED["nc"] = _build_bass()
    nc = _CACHED["nc"]

    svt = _svt_sym(inp['sv'])
    Ks = _assemble_kernel_sym(inp)
    WA, W2 = _weight_slabs(Ks)
    gam, bias = _gam_bias(inp['bn_g_s'], inp['bn_g_v'], inp['bias_s'])

    in_maps = []
    for c in range(NCORES):
        b, h = c // 2, c % 2
        c1, c2e = _core_slabs(svt, b, h)
        in_maps.append({
            "in1": c1,
            "in2e": c2e,
            "wa_in": WA,
            "w2_in": np.ascontiguousarray(W2.transpose(1, 0, 2).reshape(128, 49 * W2_COLS)),
            "gam_in": gam.reshape(64, 1),
            "bias_in": bias.reshape(64, 1),
        })

    res = run_bass_kernel_spmd(nc, in_maps, core_ids=list(range(NCORES)), trace=trace)

    y = np.zeros((B, CO, 16, 16, 16), np.float32)
    for c in range(NCORES):
        b, h = c // 2, c % 2
        y[b, :, 8 * h:8 * h + 8] = res.results[c]["out"]
    return y, res


def kernel(**inputs) -> np.ndarray:
    y, _ = run_on_hw(inputs, trace=False)
    return y


# revision 18
# speedup vs baseline: 1.3431x; 1.0156x over previous
"""SE(3)-CNN block (TensorProduct -> SE3Conv -> SE3BatchNorm -> BiasRelu) on 8 trn2 cores.

Sharding: core c = (batch b=c//2, out-x-half h=c%2). Each core computes all 64
output channels for 8 of 16 output x-planes of one batch; per-field BN second
moments are combined with a tiny [1,64] AllReduce across all 8 cores.

v2 conv strategy (vs single-plane baseline):
- Every fp32r matmul covers TWO output x-planes = one full psum bank, free
  dim 448-512 (>= 256 keeps fp32r in its fast streaming mode) and half the
  instruction count.
- No M=64 boundary singles: each chunk-1 kx pair block runs its full slot
  range (s=0..9); out-of-range tap contributions land in psum half-banks the
  evacuation never reads (slot 8 L, slot 9) fed from x-padded slab planes.
- chunk2 (channels 128:160, 4 kx taps packed per 128-row contraction) uses
  slots 10-15 (banks 5-7) plus L(0),L(1),L(6),L(7).
- Weight slab dedup: chunk-1 kx blocks stored once in column order
  [0,2 | 4,6 | 1,3 | 5] so every used pair is contiguous (11.2MB vs 24MB).
- Weight DMA on the Activation HWDGE queue, slabs + tail/stat DMAs on SP,
  evacuation adds split across Vector and GpSimd.
"""
import numpy as np
from itertools import product

# problem constants (from spec / reference)
B = 4
S_IN = 16
V_IN = 16
CO = 64          # 16 scalar + 48 vector output channels
CI = 160         # 16 s + 48 v + 96 t_sym
SIZE = 7
PAD = 3
STRIDE = 2
EPS = 1e-5
NCORES = 8
NXS = 21         # x-padded slab planes per core
NZS = 19         # z-padded: zi_slab = zi_global + 2, covering zofs in [-2, 1]
NX2 = 10         # chunk-2 even slab planes (px = 2*xi, xi 0..9)
OXC = 8          # out x-planes per core
PAIRS = [(0, 0), (0, 1), (0, 2), (1, 1), (1, 2), (2, 2)]
VAR_S_DIV = 1.0 / (B * 16 * 16 * 16)
VAR_V_DIV = 1.0 / (B * 3 * 16 * 16 * 16)

SLAB1_SHAPE = (128, NXS, 32, 2, NZS)   # [ci, px, iy, pz, zi]
SLAB2_SHAPE = (128, NX2, 32, 2, NZS)
WA_COLS = 448    # [k0|k2 | k4|k6 | k1|k3 | k5]
W2_COLS = 2 * CO


# ---------------------------------------------------------------- host prep

def _assemble_kernel_sym(inp):
    """Assemble the dense conv kernel [64, 208, 7,7,7] and symmetrize the
    t-block -> [64, 160, 7,7,7]."""
    def blk(w, basis):
        w = np.asarray(w, np.float32)
        basis = np.asarray(basis, np.float32)
        mo, mi, nb = w.shape
        do, di = basis.shape[1], basis.shape[2]
        k = np.einsum('uvb,bijxyz->uivjxyz', w, basis)
        return k.reshape(mo * do, mi * di, SIZE, SIZE, SIZE)

    row_s = np.concatenate([blk(inp['w_ss'], inp['basis_ss']),
                            blk(inp['w_sv'], inp['basis_sv']),
                            blk(inp['w_st'], inp['basis_st'])], axis=1)
    row_v = np.concatenate([blk(inp['w_vs'], inp['basis_vs']),
                            blk(inp['w_vv'], inp['basis_vv']),
                            blk(inp['w_vt'], inp['basis_vt'])], axis=1)
    K = np.concatenate([row_s, row_v], axis=0)  # [64, 208, 7,7,7]

    Ks = np.empty((CO, CI, SIZE, SIZE, SIZE), np.float32)
    Ks[:, :64] = K[:, :64]
    for u in range(16):
        for pi, (i, j) in enumerate(PAIRS):
            src = K[:, 64 + 9 * u + 3 * i + j]
            if i != j:
                src = src + K[:, 64 + 9 * u + 3 * j + i]
            Ks[:, 64 + 6 * u + pi] = src
    return Ks


def _svt_sym(sv):
    """[4,64,32,32,32] -> symmetrized tensor-product features [4,160,32,32,32]."""
    sv = np.asarray(sv, np.float32)
    s = sv[:, :S_IN]
    v = sv[:, S_IN:].reshape(B, V_IN, 3, 32, 32, 32)
    t = np.empty((B, V_IN, 6, 32, 32, 32), np.float32)
    for pi, (i, j) in enumerate(PAIRS):
        t[:, :, pi] = v[:, :, i] * v[:, :, j]
    return np.concatenate([s, v.reshape(B, 48, 32, 32, 32),
                           t.reshape(B, 96, 32, 32, 32)], axis=1)


def _core_slabs(svt, b, h):
    """x/z zero-padded, z-parity-split slabs for core (b, h).

    c1 [128, 23, 32, 2, 19]: plane px holds global ix = px + 16h - 3.
    c2e [128, 10, 32, 2, 19]: block a (rows 32a:32a+32) of plane xi holds
    chunk-2 channels at ix = 2*xi + a + 16h - 3. zi_slab = zi_global + 2.
    """
    sp = svt[b].reshape(CI, 32, 32, 16, 2)   # (ci, x, y, zi, pz); iz = 2*zi + pz
    sp = np.moveaxis(sp, 4, 3)               # (ci, x, y, pz, zi)
    x0 = 16 * h - 3
    c1 = np.zeros(SLAB1_SHAPE, np.float32)
    lo, hi = max(0, x0), min(32, x0 + NXS)
    c1[:, lo - x0:hi - x0, :, :, 2:18] = sp[:128, lo:hi]
    c2e = np.zeros(SLAB2_SHAPE, np.float32)
    for a in range(4):
        for xi in range(NX2):
            ix = 2 * xi + a + x0
            if 0 <= ix < 32:
                c2e[32 * a:32 * a + 32, xi, :, :, 2:18] = sp[128:160, ix]
    return c1, c2e


def _weight_slabs(Ks):
    """(WA, W2). WA [49, 128, 448]: chunk-1 taps in column order
    [k0|k2|k4|k6|k1|k3|k5] (64 cols each); pairs (0,2),(4,6),(1,3) are the
    contiguous 128-col windows at 0, 128, 256; k5 singles at 384.
    W2 [49, 128, 128]: 4-way kx-merged chunk-2 (g=0: kx 0..3 lower 64 cols,
    g=1: kx 4..6 upper)."""
    KXORD = [0, 2, 4, 6, 1, 3, 5]
    WA = np.zeros((49, 128, WA_COLS), np.float32)
    W2 = np.zeros((49, 128, W2_COLS), np.float32)
    for ky, kz in product(range(SIZE), range(SIZE)):
        i = ky * SIZE + kz
        for ci, kx in enumerate(KXORD):
            WA[i, :, 64 * ci:64 * ci + 64] = Ks[:, :128, kx, ky, kz].T
        for g in range(2):
            for a in range(4):
                kx = 4 * g + a
                if kx > 6:
                    continue
                W2[i, 32 * a:32 * a + 32, 64 * g:64 * (g + 1)] = \
                    Ks[:, 128:160, kx, ky, kz].T
    return WA, W2


def _gam_bias(bn_g_s, bn_g_v, bias_s):
    """Per-channel gamma [64] (vector gammas replicated x3) and bias [64]."""
    gam = np.empty(64, np.float32)
    gam[:16] = np.asarray(bn_g_s, np.float32)
    gam[16:] = np.repeat(np.asarray(bn_g_v, np.float32), 3)
    bias = np.zeros(64, np.float32)
    bias[:16] = np.asarray(bias_s, np.float32)
    return gam, bias


# ---------------------------------------------------------------- matmul plan

def _box(ky, kz):
    """Valid output range + slab coords for kernel offsets (ky, kz)."""
    d = kz - 3
    p = d % 2
    zofs = (d - p) // 2
    oy0 = max(0, (4 - ky) // 2)
    oy1 = min(16, (34 - ky) // 2 + 1)
    iy0 = 2 * oy0 + ky - 3
    return dict(p=p, zs=zofs + 2, iy0=iy0, oyc=oy1 - oy0, oy0=oy0)


def _mm_plan():
    """Matmul descriptors (src, i, wc, ww, x0, bank) in issue order.

    Each matmul writes one full psum bank `bank` = slots (2*bank, 2*bank+1):
    partitions 0:ww x 2 halves x [oy0:oy0+oyc] x 16. Slot s holds plane s at
    partitions 0:64 (L) and plane s-1 at 64:128 (U). Chunk-2 c2p uses slots
    10..15 (banks 5-7): slot 10+j-2 L = plane j lower-taps, slot 10+j U =
    plane j upper-taps. Slot 8 L / slot 9 / U(0) are garbage sinks.

    Order: chunk-2 first (its half-size slab loads fast and its compute
    covers the big chunk-1 slab's DMA), then chunk-1.
    """
    plan = []
    # chunk-2 as kind-major sweeps: sweep k needs only slab planes 2k:2k+2,
    # so compute starts as soon as the first planes land and each later
    # sweep finds its planes already resident (w2 is fully resident)
    for i in range(49):
        plan.append(('c2', i, 0, 64, 0, 0, 2))     # c2s g0 -> L(0),L(1)
    for i in range(49):
        plan.append(('c2', i, 0, 128, 2, 5, 2))    # c2p -> slots (10,11)
    for i in range(49):
        plan.append(('c2', i, 0, 128, 4, 6, 2))    # slots (12,13)
    for i in range(49):
        plan.append(('c2', i, 0, 128, 6, 7, 2))    # slots (14,15)
    for i in range(49):
        plan.append(('c2', i, 64, 64, 8, 3, 2))    # c2s g1 -> L(6),L(7)
    for i in range(49):
        for wc, ka in ((0, 0), (128, 4), (256, 1)):  # P(0,2), P(4,6), P(1,3)
            for bank in range(4):
                plan.append(('c1', i, wc, 128, ka + 4 * bank, bank, 2))
            # slot 9 is pure garbage, so the (s8,s9) pair shrinks to a
            # single-plane s8 matmul (only its U half, plane 7, is real)
            plan.append(('c1', i, wc, 128, ka + 16, 4, 1))
        for bank in range(4):                       # S5: px (5+4b), L only
            plan.append(('c1', i, 384, 64, 5 + 4 * bank, bank, 2))
    return plan


_PLAN = _mm_plan()

# stop flags: last matmul touching each psum bank
_LAST_IDX = {}
for _n, _d in enumerate(_PLAN):
    _LAST_IDX[_d[5]] = _n
_STOPS = set(_LAST_IDX.values())


# ---------------------------------------------------------------- numpy shadow

def _shadow_core(c1, c2e, WA, W2):
    """Execute the matmul plan in numpy. Returns conv output [64, 8, 16, 16]."""
    banks = np.zeros((8, 128, 2, 16, 16), np.float32)
    for src, i, wc, ww, x0, bank, npos in _PLAN:
        ky, kz = i // 7, i % 7
        bx = _box(ky, kz)
        sl = c2e if src == 'c2' else c1
        step = 1 if src == 'c2' else 2
        rhs = sl[:, x0:x0 + (npos - 1) * step + 1:step,
                 bx['iy0']:bx['iy0'] + 2 * bx['oyc']:2, bx['p'],
                 bx['zs']:bx['zs'] + 16]
        lhsT = (W2 if src == 'c2' else WA)[i][:, wc:wc + ww]
        contrib = np.einsum('km,kpbc->mpbc', lhsT, rhs)
        banks[bank][:ww, 0:npos, bx['oy0']:bx['oy0'] + bx['oyc'], :] += contrib
    out = np.empty((OXC, CO, 16, 16), np.float32)
    for j in range(OXC):
        acc = banks[j // 2][0:64, j % 2].copy()        # L(j)
        su = j + 1
        acc = acc + banks[su // 2][64:128, su % 2]     # U(j+1)
        if j >= 2:
            s = 8 + j                                  # LB: slot 10+j-2
            acc = acc + banks[s // 2][0:64, s % 2]
        if j <= 5:
            s = 10 + j                                 # UB: slot 10+j
            acc = acc + banks[s // 2][64:128, s % 2]
        out[j] = acc
    return out.transpose(1, 0, 2, 3)


def shadow_forward(inp):
    """Full-model numpy shadow of the device computation (for plan validation)."""
    svt = _svt_sym(inp['sv'])
    Ks = _assemble_kernel_sym(inp)
    WA, W2 = _weight_slabs(Ks)
    gam, bias = _gam_bias(inp['bn_g_s'], inp['bn_g_v'], inp['bias_s'])

    y = np.zeros((B, CO, 16, 16, 16), np.float32)
    ss = np.zeros(64, np.float64)
    for c in range(NCORES):
        b, h = c // 2, c % 2
        c1, c2e = _core_slabs(svt, b, h)
        out = _shadow_core(c1, c2e, WA, W2)
        y[b, :, 8 * h:8 * h + 8] = out
        ss += (out.astype(np.float64) ** 2).sum(axis=(1, 2, 3))

    var = np.empty(64)
    var[:16] = ss[:16] * VAR_S_DIV
    vv = (ss[16::3] + ss[17::3] + ss[18::3]) * VAR_V_DIV
    var[16:] = np.repeat(vv, 3)
    scale = gam / np.sqrt(var + EPS)
    y = y * scale[None, :, None, None, None].astype(np.float32)
    y[:, :16] = np.maximum(y[:, :16] + bias[:16][None, :, None, None, None], 0.0)
    return y


# ---------------------------------------------------------------- bass kernel

_CACHED = {}


def _build_bass():
    import concourse.bass as bass
    import concourse.tile as tile
    import concourse.mybir as mybir
    from concourse import bacc

    f32 = mybir.dt.float32
    f32r = mybir.dt.float32r
    bf16 = mybir.dt.bfloat16

    nc = bacc.Bacc("TRN2", target_bir_lowering=False, debug=False, num_devices=NCORES)

    in1 = nc.dram_tensor("in1", list(SLAB1_SHAPE), f32r, kind="ExternalInput").ap()
    in2e = nc.dram_tensor("in2e", list(SLAB2_SHAPE), f32r, kind="ExternalInput").ap()
    wa_in = nc.dram_tensor("wa_in", [49, 128, WA_COLS], f32r, kind="ExternalInput").ap()
    w2_in = nc.dram_tensor("w2_in", [128, 49 * W2_COLS], f32r, kind="ExternalInput").ap()
    gam_in = nc.dram_tensor("gam_in", [64, 1], f32, kind="ExternalInput").ap()
    bias_in = nc.dram_tensor("bias_in", [64, 1], f32, kind="ExternalInput").ap()
    out_d = nc.dram_tensor("out", [CO, OXC, 16, 16], f32, kind="ExternalOutput").ap()

    with tile.TileContext(nc) as tc:
        with (
            tc.tile_pool(name="slab", bufs=1) as slab_pool,
            tc.tile_pool(name="wp", bufs=4) as wpool,
            tc.tile_pool(name="ps", bufs=1, space="PSUM") as ps,
            tc.tile_pool(name="outp", bufs=1) as outp,
            tc.tile_pool(name="stat", bufs=1) as stat,
            tc.tile_pool(name="dram", bufs=1, space="DRAM") as dram,
        ):
            # 8 psum banks = 16 half-bank slots (garbage: 8L, 9, U(0))
            pq = [ps.tile([128, 2, 16, 16], f32, tag=f"pq{t}", name=f"pq{t}")
                  for t in range(8)]

            # tiny BN params first on the SP queue so they're resident early
            gam_t = stat.tile([CO, 1], f32, tag="gam")
            bias_t = stat.tile([CO, 1], f32, tag="bias")
            nc.sync.dma_start(gam_t[:], gam_in[:])
            nc.sync.dma_start(bias_t[:], bias_in[:])

            # slabs in bf16 (fp32r weights keep the contraction accurate; the
            # halved moving-operand footprint relieves SBUF read pressure and
            # halves slab DMA). Split across both HWDGE rings: SP carries the
            # early chunk-2 planes + chunk-1, Activation the late c2 planes.
            sl2 = slab_pool.tile(list(SLAB2_SHAPE), f32r, tag="slab2",
                                 name="slab_c2")
            sl1 = slab_pool.tile(list(SLAB1_SHAPE), f32r, tag="slab",
                                 name="slab_c1")
            nc.sync.dma_start(sl2[:, 0:2], in2e[:, 0:2])
            w2all = wpool.tile([128, 49 * W2_COLS], f32r, tag="w2a",
                               name="w2all", bufs=1)
            nc.sync.dma_start(w2all[:, :24 * W2_COLS], w2_in[:, :24 * W2_COLS])
            nc.sync.dma_start(w2all[:, 24 * W2_COLS:], w2_in[:, 24 * W2_COLS:])

            # start=True clears the WHOLE psum bank, so open each bank once
            # with a zero-weight full-bank matmul (also a WAW dep that orders
            # it before every accumulate); all real matmuls use start=False.
            # rhs is a memset zeros tile so the opens run before any DMA lands.
            zw_f = stat.tile([128, 128], f32, tag="zw")
            zr_f = stat.tile([128, 512], f32, tag="zr")
            nc.vector.memset(zw_f[:], 0.0)
            nc.vector.memset(zr_f[:], 0.0)
            zw = zw_f.bitcast(f32r)
            zr = zr_f.bitcast(f32r)
            for t in range(8):
                nc.tensor.matmul(pq[t].rearrange("c a y z -> c (a y z)"),
                                 zw[:], zr[:], start=True, stop=False)

            # all chunk-2 weights live in ONE resident tile: a [128 x 25KB]
            # DMA is 128 large descriptors (descriptor-efficient), so chunk-2
            # never starves; chunk-1 tiles trickle on the slow Activation
            # ring, which keeps bulk SBUF writes from contending with the
            # fast-mode matmul stream
            for x in range(2, 10, 2):
                nc.sync.dma_start(sl2[:, x:x + 2], in2e[:, x:x + 2])
            nc.sync.dma_start(sl1[:, 0:11], in1[:, 0:11])
            nc.sync.dma_start(sl1[:, 11:NXS], in1[:, 11:NXS])
            wat = {}
            for i in range(49):
                w = wpool.tile([128, WA_COLS], f32r, tag="wa", name=f"wa_{i}",
                               bufs=4)
                nc.scalar.dma_start(w[:], wa_in[i])
                wat[i] = w


            for n, (src, i, wc, ww, x0, bank, npos) in enumerate(_PLAN):
                ky, kz = i // 7, i % 7
                bx = _box(ky, kz)
                if src == 'c2':
                    lhsT = w2all[:, i * W2_COLS + wc:i * W2_COLS + wc + ww]
                    sl, step = sl2, 1
                else:
                    lhsT = wat[i][:, wc:wc + ww]
                    sl, step = sl1, 2
                yslice = slice(bx['iy0'], bx['iy0'] + 2 * bx['oyc'] - 1, 2)
                if npos == 2:
                    rhs = sl[:, x0:x0 + step + 1:step, yslice, bx['p'],
                             bx['zs']:bx['zs'] + 16]
                    out_ap = pq[bank][0:ww, 0:2,
                                      bx['oy0']:bx['oy0'] + bx['oyc'], :]
                else:
                    rhs = sl[:, x0, yslice, bx['p'], bx['zs']:bx['zs'] + 16]
                    out_ap = pq[bank][0:ww, 0,
                                      bx['oy0']:bx['oy0'] + bx['oyc'], :]
                nc.tensor.matmul(out_ap, lhsT, rhs,
                                 start=False, stop=n in _STOPS)

            # evacuate: plane j = L(j) + U(j+1) [+ LB(10+j-2) j>=2]
            #                                   [+ UB(10+j)   j<=5]
            # a TensorTensor may read only ONE psum operand: Scalar engine
            # copies psum->sbuf, Vector accumulates the second psum operand
            osb = outp.tile([CO, OXC, 16, 16], f32, tag="osb")
            usb = outp.tile([128, OXC, 16, 16], f32, tag="usb")
            for j in range(OXC):
                nc.scalar.activation(osb[:, j], pq[j // 2][0:64, j % 2],
                                     mybir.ActivationFunctionType.Copy,
                                     scale=1.0)
                if j >= 2:
                    s = 8 + j
                    nc.vector.tensor_add(osb[:, j], osb[:, j],
                                         pq[s // 2][0:64, s % 2])
                su = j + 1
                nc.scalar.activation(usb[64:128, j],
                                     pq[su // 2][64:128, su % 2],
                                     mybir.ActivationFunctionType.Copy,
                                     scale=1.0)
                if j <= 5:
                    s = 10 + j
                    nc.vector.tensor_add(usb[64:128, j], usb[64:128, j],
                                         pq[s // 2][64:128, s % 2])
            # rendezvous: a 4-byte token dependent on the first evacuated
            # plane enters a dummy AllReduce, aligning the cores while the
            # evac/stats chain runs, so the real AllReduce's peer wait
            # overlaps work instead of extending the tail
            cc_b_in = dram.tile([1, 64], f32, tag="ccbi")
            cc_b_out = dram.tile([1, 64], f32, tag="ccbo")
            nc.sync.dma_start(cc_b_in[0:1, 0:1], osb[0:1, 0, 0, 0:1])
            nc.gpsimd.collective_compute(
                "AllReduce", mybir.AluOpType.add,
                replica_groups=[list(range(NCORES))],
                ins=[cc_b_in.opt()], outs=[cc_b_out.opt()],
            )

            # move upper-half partials down to partitions 0:64 and add
            u_dram = dram.tile([64, OXC, 16, 16], f32, tag="ud")
            nc.sync.dma_start(u_dram[:], usb[64:128])
            nc.sync.dma_start(usb[0:64], u_dram[:])
            of = osb.rearrange("c x y z -> c (x y z)")
            uf = usb.rearrange("c x y z -> c (x y z)")
            nc.vector.tensor_add(of[:, :], of[:, :], uf[0:64, :])

            # per-channel sum of squares in ONE scalar-engine op (Square with
            # free-axis accumulator) -> local variance contribution
            # (linear in the sums, so the AllReduce can carry variance
            # directly and the post-collective chain stays short)
            sq = outp.tile([CO, 2048], f32, tag="sq")
            ssq = stat.tile([CO, 1], f32, tag="ssq")
            nc.scalar.activation(sq[:], of[:, :],
                                 mybir.ActivationFunctionType.Square,
                                 scale=1.0, accum_out=ssq[:, :])
            ss_row = stat.tile([1, 64], f32, tag="ssrow")
            vloc = stat.tile([1, 64], f32, tag="vloc")
            tmp16 = stat.tile([1, 16], f32, tag="tmp16")
            ss_dram = dram.tile([1, 64], f32, tag="ssd")
            nc.sync.dma_start(ss_dram[0, :], ssq[:, 0])
            nc.sync.dma_start(ss_row[:], ss_dram[:])
            nc.vector.tensor_add(tmp16[:], ss_row[:, 16::3], ss_row[:, 17::3])
            nc.vector.tensor_add(tmp16[:], tmp16[:], ss_row[:, 18::3])
            nc.vector.tensor_scalar_mul(vloc[:, 0:16], ss_row[:, 0:16], VAR_S_DIV)
            for j in range(3):
                nc.vector.tensor_scalar_mul(vloc[:, 16 + j::3], tmp16[:], VAR_V_DIV)

            v_dram = dram.tile([1, 64], f32, tag="vd")
            v_red = dram.tile([1, 64], f32, tag="vr")
            nc.sync.dma_start(v_dram[:], vloc[:])
            nc.gpsimd.collective_compute(
                "AllReduce", mybir.AluOpType.add,
                replica_groups=[list(range(NCORES))],
                ins=[v_dram.opt()], outs=[v_red.opt()],
            )

            # scale = gamma / sqrt(var + eps), in per-partition layout
            var_col = stat.tile([CO, 1], f32, tag="varcol")
            nc.sync.dma_start(var_col[:, 0], v_red[0, :])
            eps_t = stat.tile([CO, 1], f32, tag="eps")
            nc.vector.memset(eps_t[:], EPS)
            sd = stat.tile([CO, 1], f32, tag="sd")
            nc.scalar.activation(sd[:], var_col[:], mybir.ActivationFunctionType.Sqrt,
                                 bias=eps_t[:], scale=1.0)
            inv = stat.tile([CO, 1], f32, tag="inv")
            nc.vector.reciprocal(inv[:], sd[:])
            scale_col = stat.tile([CO, 1], f32, tag="sccol")
            nc.vector.tensor_mul(scale_col[:], inv[:], gam_t[:])

            # apply BN scale everywhere, then bias+relu on scalar channels;
            # ship the vector channels while the relu runs
            nc.vector.tensor_scalar_mul(of[:, :], of[:, :], scale_col[:, :])
            nc.scalar.activation(of[0:16, :], of[0:16, :],
                                 mybir.ActivationFunctionType.Relu,
                                 bias=bias_t[0:16, :], scale=1.0)
            nc.sync.dma_start(out_d[16:64], osb[16:64])
            nc.sync.dma_start(out_d[0:16], osb[0:16])

    nc.compile()
    return nc


def _install_ntff_hook():
    import sys, types
    if "antenv.axon_hooks" in sys.modules:
        return
    mod = types.ModuleType("antenv.axon_hooks")
    mod._hook = None
    mod.set_axon_ntff_profile_hook = lambda h: setattr(mod, "_hook", h)
    mod.get_axon_ntff_profile_hook = lambda: mod._hook
    sys.modules["antenv.axon_hooks"] = mod
    try:
        import antenv
        antenv.axon_hooks = mod
        from trn_agent_boot.trn_boot import _ntff_profile_via_ctypes
        mod.set_axon_ntff_profile_hook(_ntff_profile_via_ctypes("/opt/axon/libaxon_pjrt.so"))
    except Exception:
        pass


def run_on_hw(inp, trace=False):
    """Run the kernel on 8 cores. Returns (full output [4,64,16,16,16], results)."""
    from concourse.bass_utils import run_bass_kernel_spmd

    if "nc" not in _CACHED:
        _install_ntff_hook()
        _CACHED["nc"] = _build_bass()
    nc = _CACHED["nc"]

    svt = _svt_sym(inp['sv'])
    Ks = _assemble_kernel_sym(inp)
    WA, W2 = _weight_slabs(Ks)
    gam, bias = _gam_bias(inp['bn_g_s'], inp['bn_g_v'], inp['bias_s'])

    in_maps = []
    for c in range(NCORES):
        b, h = c // 2, c % 2
        c1, c2e = _core_slabs(svt, b, h)
        in_maps.append({
            "in1": c1,
            "in2e": c2e,
            "wa_in": WA,
            "w2_in": np.ascontiguousarray(W2.transpose(1, 0, 2).reshape(128, 49 * W2_COLS)),
            "gam_in": gam.reshape(64, 1),
            "bias_in": bias.reshape(64, 1),
        })

    res = run_bass_kernel_spmd(nc, in_maps, core_ids=list(range(NCORES)), trace=trace)

    y = np.zeros((B, CO, 16, 16, 16), np.float32)
    for c in range(NCORES):
        b, h = c // 2, c % 2
        y[b, :, 8 * h:8 * h + 8] = res.results[c]["out"]
    return y, res


def kernel(**inputs) -> np.ndarray:
    y, _ = run_on_hw(inputs, trace=False)
    return y


# revision 19
# speedup vs baseline: 1.3660x; 1.0171x over previous
"""SE(3)-CNN block (TensorProduct -> SE3Conv -> SE3BatchNorm -> BiasRelu) on 8 trn2 cores.

Sharding: core c = (batch b=c//2, out-x-half h=c%2). Each core computes all 64
output channels for 8 of 16 output x-planes of one batch; per-field BN second
moments are combined with a tiny [1,64] AllReduce across all 8 cores.

v2 conv strategy (vs single-plane baseline):
- Every fp32r matmul covers TWO output x-planes = one full psum bank, free
  dim 448-512 (>= 256 keeps fp32r in its fast streaming mode) and half the
  instruction count.
- No M=64 boundary singles: each chunk-1 kx pair block runs its full slot
  range (s=0..9); out-of-range tap contributions land in psum half-banks the
  evacuation never reads (slot 8 L, slot 9) fed from x-padded slab planes.
- chunk2 (channels 128:160, 4 kx taps packed per 128-row contraction) uses
  slots 10-15 (banks 5-7) plus L(0),L(1),L(6),L(7).
- Weight slab dedup: chunk-1 kx blocks stored once in column order
  [0,2 | 4,6 | 1,3 | 5] so every used pair is contiguous (11.2MB vs 24MB).
- Weight DMA on the Activation HWDGE queue, slabs + tail/stat DMAs on SP,
  evacuation adds split across Vector and GpSimd.
"""
import numpy as np
from itertools import product

# problem constants (from spec / reference)
B = 4
S_IN = 16
V_IN = 16
CO = 64          # 16 scalar + 48 vector output channels
CI = 160         # 16 s + 48 v + 96 t_sym
SIZE = 7
PAD = 3
STRIDE = 2
EPS = 1e-5
NCORES = 8
NXS = 21         # x-padded slab planes per core
NZS = 19         # z-padded: zi_slab = zi_global + 2, covering zofs in [-2, 1]
NX2 = 10         # chunk-2 even slab planes (px = 2*xi, xi 0..9)
OXC = 8          # out x-planes per core
PAIRS = [(0, 0), (0, 1), (0, 2), (1, 1), (1, 2), (2, 2)]
VAR_S_DIV = 1.0 / (B * 16 * 16 * 16)
VAR_V_DIV = 1.0 / (B * 3 * 16 * 16 * 16)

SLAB1_SHAPE = (128, NXS, 32, 2, NZS)   # [ci, px, iy, pz, zi]
SLAB2_SHAPE = (128, NX2, 32, 2, NZS)
WA_COLS = 448    # [k0|k2 | k4|k6 | k1|k3 | k5]
W2_COLS = 2 * CO


# ---------------------------------------------------------------- host prep

def _assemble_kernel_sym(inp):
    """Assemble the dense conv kernel [64, 208, 7,7,7] and symmetrize the
    t-block -> [64, 160, 7,7,7]."""
    def blk(w, basis):
        w = np.asarray(w, np.float32)
        basis = np.asarray(basis, np.float32)
        mo, mi, nb = w.shape
        do, di = basis.shape[1], basis.shape[2]
        k = np.einsum('uvb,bijxyz->uivjxyz', w, basis)
        return k.reshape(mo * do, mi * di, SIZE, SIZE, SIZE)

    row_s = np.concatenate([blk(inp['w_ss'], inp['basis_ss']),
                            blk(inp['w_sv'], inp['basis_sv']),
                            blk(inp['w_st'], inp['basis_st'])], axis=1)
    row_v = np.concatenate([blk(inp['w_vs'], inp['basis_vs']),
                            blk(inp['w_vv'], inp['basis_vv']),
                            blk(inp['w_vt'], inp['basis_vt'])], axis=1)
    K = np.concatenate([row_s, row_v], axis=0)  # [64, 208, 7,7,7]

    Ks = np.empty((CO, CI, SIZE, SIZE, SIZE), np.float32)
    Ks[:, :64] = K[:, :64]
    for u in range(16):
        for pi, (i, j) in enumerate(PAIRS):
            src = K[:, 64 + 9 * u + 3 * i + j]
            if i != j:
                src = src + K[:, 64 + 9 * u + 3 * j + i]
            Ks[:, 64 + 6 * u + pi] = src
    return Ks


def _svt_sym(sv):
    """[4,64,32,32,32] -> symmetrized tensor-product features [4,160,32,32,32]."""
    sv = np.asarray(sv, np.float32)
    s = sv[:, :S_IN]
    v = sv[:, S_IN:].reshape(B, V_IN, 3, 32, 32, 32)
    t = np.empty((B, V_IN, 6, 32, 32, 32), np.float32)
    for pi, (i, j) in enumerate(PAIRS):
        t[:, :, pi] = v[:, :, i] * v[:, :, j]
    return np.concatenate([s, v.reshape(B, 48, 32, 32, 32),
                           t.reshape(B, 96, 32, 32, 32)], axis=1)


def _core_slabs(svt, b, h):
    """x/z zero-padded, z-parity-split slabs for core (b, h).

    c1 [128, 23, 32, 2, 19]: plane px holds global ix = px + 16h - 3.
    c2e [128, 10, 32, 2, 19]: block a (rows 32a:32a+32) of plane xi holds
    chunk-2 channels at ix = 2*xi + a + 16h - 3. zi_slab = zi_global + 2.
    """
    sp = svt[b].reshape(CI, 32, 32, 16, 2)   # (ci, x, y, zi, pz); iz = 2*zi + pz
    sp = np.moveaxis(sp, 4, 3)               # (ci, x, y, pz, zi)
    x0 = 16 * h - 3
    c1 = np.zeros(SLAB1_SHAPE, np.float32)
    lo, hi = max(0, x0), min(32, x0 + NXS)
    c1[:, lo - x0:hi - x0, :, :, 2:18] = sp[:128, lo:hi]
    c2e = np.zeros(SLAB2_SHAPE, np.float32)
    for a in range(4):
        for xi in range(NX2):
            ix = 2 * xi + a + x0
            if 0 <= ix < 32:
                c2e[32 * a:32 * a + 32, xi, :, :, 2:18] = sp[128:160, ix]
    return c1, c2e


def _weight_slabs(Ks):
    """(WA, W2). WA [49, 128, 448]: chunk-1 taps in column order
    [k0|k2|k4|k6|k1|k3|k5] (64 cols each); pairs (0,2),(4,6),(1,3) are the
    contiguous 128-col windows at 0, 128, 256; k5 singles at 384.
    W2 [49, 128, 128]: 4-way kx-merged chunk-2 (g=0: kx 0..3 lower 64 cols,
    g=1: kx 4..6 upper)."""
    KXORD = [0, 2, 4, 6, 1, 3, 5]
    WA = np.zeros((49, 128, WA_COLS), np.float32)
    W2 = np.zeros((49, 128, W2_COLS), np.float32)
    for ky, kz in product(range(SIZE), range(SIZE)):
        i = ky * SIZE + kz
        for ci, kx in enumerate(KXORD):
            WA[i, :, 64 * ci:64 * ci + 64] = Ks[:, :128, kx, ky, kz].T
        for g in range(2):
            for a in range(4):
                kx = 4 * g + a
                if kx > 6:
                    continue
                W2[i, 32 * a:32 * a + 32, 64 * g:64 * (g + 1)] = \
                    Ks[:, 128:160, kx, ky, kz].T
    return WA, W2


def _gam_bias(bn_g_s, bn_g_v, bias_s):
    """Per-channel gamma [64] (vector gammas replicated x3) and bias [64]."""
    gam = np.empty(64, np.float32)
    gam[:16] = np.asarray(bn_g_s, np.float32)
    gam[16:] = np.repeat(np.asarray(bn_g_v, np.float32), 3)
    bias = np.zeros(64, np.float32)
    bias[:16] = np.asarray(bias_s, np.float32)
    return gam, bias


# ---------------------------------------------------------------- matmul plan

def _box(ky, kz):
    """Valid output range + slab coords for kernel offsets (ky, kz)."""
    d = kz - 3
    p = d % 2
    zofs = (d - p) // 2
    oy0 = max(0, (4 - ky) // 2)
    oy1 = min(16, (34 - ky) // 2 + 1)
    iy0 = 2 * oy0 + ky - 3
    return dict(p=p, zs=zofs + 2, iy0=iy0, oyc=oy1 - oy0, oy0=oy0)


def _mm_plan():
    """Matmul descriptors (src, i, wc, ww, x0, bank) in issue order.

    Each matmul writes one full psum bank `bank` = slots (2*bank, 2*bank+1):
    partitions 0:ww x 2 halves x [oy0:oy0+oyc] x 16. Slot s holds plane s at
    partitions 0:64 (L) and plane s-1 at 64:128 (U). Chunk-2 c2p uses slots
    10..15 (banks 5-7): slot 10+j-2 L = plane j lower-taps, slot 10+j U =
    plane j upper-taps. Slot 8 L / slot 9 / U(0) are garbage sinks.

    Order: chunk-2 first (its half-size slab loads fast and its compute
    covers the big chunk-1 slab's DMA), then chunk-1.
    """
    plan = []
    # chunk-2 as kind-major sweeps: sweep k needs only slab planes 2k:2k+2,
    # so compute starts as soon as the first planes land and each later
    # sweep finds its planes already resident (w2 is fully resident)
    for i in range(49):
        plan.append(('c2', i, 0, 64, 0, 0, 2))     # c2s g0 -> L(0),L(1)
    for i in range(49):
        plan.append(('c2', i, 0, 128, 2, 5, 2))    # c2p -> slots (10,11)
    for i in range(49):
        plan.append(('c2', i, 0, 128, 4, 6, 2))    # slots (12,13)
    for i in range(49):
        plan.append(('c2', i, 0, 128, 6, 7, 2))    # slots (14,15)
    for i in range(49):
        plan.append(('c2', i, 64, 64, 8, 3, 2))    # c2s g1 -> L(6),L(7)
    for i in range(49):
        for wc, ka in ((0, 0), (128, 4), (256, 1)):  # P(0,2), P(4,6), P(1,3)
            for bank in range(4):
                plan.append(('c1', i, wc, 128, ka + 4 * bank, bank, 2))
            # slot 9 is pure garbage, so the (s8,s9) pair shrinks to a
            # single-plane s8 matmul (only its U half, plane 7, is real)
            plan.append(('c1', i, wc, 128, ka + 16, 4, 1))
        for bank in range(4):                       # S5: px (5+4b), L only
            plan.append(('c1', i, 384, 64, 5 + 4 * bank, bank, 2))
    return plan


_PLAN = _mm_plan()

# stop flags: last matmul touching each psum bank
_LAST_IDX = {}
for _n, _d in enumerate(_PLAN):
    _LAST_IDX[_d[5]] = _n
_STOPS = set(_LAST_IDX.values())


# ---------------------------------------------------------------- numpy shadow

def _shadow_core(c1, c2e, WA, W2):
    """Execute the matmul plan in numpy. Returns conv output [64, 8, 16, 16]."""
    banks = np.zeros((8, 128, 2, 16, 16), np.float32)
    for src, i, wc, ww, x0, bank, npos in _PLAN:
        ky, kz = i // 7, i % 7
        bx = _box(ky, kz)
        sl = c2e if src == 'c2' else c1
        step = 1 if src == 'c2' else 2
        rhs = sl[:, x0:x0 + (npos - 1) * step + 1:step,
                 bx['iy0']:bx['iy0'] + 2 * bx['oyc']:2, bx['p'],
                 bx['zs']:bx['zs'] + 16]
        lhsT = (W2 if src == 'c2' else WA)[i][:, wc:wc + ww]
        contrib = np.einsum('km,kpbc->mpbc', lhsT, rhs)
        banks[bank][:ww, 0:npos, bx['oy0']:bx['oy0'] + bx['oyc'], :] += contrib
    out = np.empty((OXC, CO, 16, 16), np.float32)
    for j in range(OXC):
        acc = banks[j // 2][0:64, j % 2].copy()        # L(j)
        su = j + 1
        acc = acc + banks[su // 2][64:128, su % 2]     # U(j+1)
        if j >= 2:
            s = 8 + j                                  # LB: slot 10+j-2
            acc = acc + banks[s // 2][0:64, s % 2]
        if j <= 5:
            s = 10 + j                                 # UB: slot 10+j
            acc = acc + banks[s // 2][64:128, s % 2]
        out[j] = acc
    return out.transpose(1, 0, 2, 3)


def shadow_forward(inp):
    """Full-model numpy shadow of the device computation (for plan validation)."""
    svt = _svt_sym(inp['sv'])
    Ks = _assemble_kernel_sym(inp)
    WA, W2 = _weight_slabs(Ks)
    gam, bias = _gam_bias(inp['bn_g_s'], inp['bn_g_v'], inp['bias_s'])

    y = np.zeros((B, CO, 16, 16, 16), np.float32)
    ss = np.zeros(64, np.float64)
    for c in range(NCORES):
        b, h = c // 2, c % 2
        c1, c2e = _core_slabs(svt, b, h)
        out = _shadow_core(c1, c2e, WA, W2)
        y[b, :, 8 * h:8 * h + 8] = out
        ss += (out.astype(np.float64) ** 2).sum(axis=(1, 2, 3))

    var = np.empty(64)
    var[:16] = ss[:16] * VAR_S_DIV
    vv = (ss[16::3] + ss[17::3] + ss[18::3]) * VAR_V_DIV
    var[16:] = np.repeat(vv, 3)
    scale = gam / np.sqrt(var + EPS)
    y = y * scale[None, :, None, None, None].astype(np.float32)
    y[:, :16] = np.maximum(y[:, :16] + bias[:16][None, :, None, None, None], 0.0)
    return y


# ---------------------------------------------------------------- bass kernel

_CACHED = {}


def _build_bass():
    import concourse.bass as bass
    import concourse.tile as tile
    import concourse.mybir as mybir
    from concourse import bacc

    f32 = mybir.dt.float32
    f32r = mybir.dt.float32r
    bf16 = mybir.dt.bfloat16

    nc = bacc.Bacc("TRN2", target_bir_lowering=False, debug=False, num_devices=NCORES)

    in1 = nc.dram_tensor("in1", list(SLAB1_SHAPE), f32r, kind="ExternalInput").ap()
    in2e = nc.dram_tensor("in2e", list(SLAB2_SHAPE), f32r, kind="ExternalInput").ap()
    wa_in = nc.dram_tensor("wa_in", [49, 128, WA_COLS], f32r, kind="ExternalInput").ap()
    w2_in = nc.dram_tensor("w2_in", [128, 49 * W2_COLS], f32r, kind="ExternalInput").ap()
    fold_in = nc.dram_tensor("fold_in", [128, 64], f32, kind="ExternalInput").ap()
    gam_in = nc.dram_tensor("gam_in", [64, 1], f32, kind="ExternalInput").ap()
    bias_in = nc.dram_tensor("bias_in", [64, 1], f32, kind="ExternalInput").ap()
    out_d = nc.dram_tensor("out", [CO, OXC, 16, 16], f32, kind="ExternalOutput").ap()

    with tile.TileContext(nc) as tc:
        with (
            tc.tile_pool(name="slab", bufs=1) as slab_pool,
            tc.tile_pool(name="wp", bufs=4) as wpool,
            tc.tile_pool(name="ps", bufs=1, space="PSUM") as ps,
            tc.tile_pool(name="outp", bufs=1) as outp,
            tc.tile_pool(name="stat", bufs=1) as stat,
            tc.tile_pool(name="dram", bufs=1, space="DRAM") as dram,
        ):
            # 8 psum banks = 16 half-bank slots (garbage: 8L, 9, U(0))
            pq = [ps.tile([128, 2, 16, 16], f32, tag=f"pq{t}", name=f"pq{t}")
                  for t in range(8)]

            # tiny BN params first on the SP queue so they're resident early
            gam_t = stat.tile([CO, 1], f32, tag="gam")
            bias_t = stat.tile([CO, 1], f32, tag="bias")
            fold_t = stat.tile([128, 64], f32, tag="fold")
            nc.sync.dma_start(fold_t[:], fold_in[:])
            nc.sync.dma_start(gam_t[:], gam_in[:])
            nc.sync.dma_start(bias_t[:], bias_in[:])

            # slabs in bf16 (fp32r weights keep the contraction accurate; the
            # halved moving-operand footprint relieves SBUF read pressure and
            # halves slab DMA). Split across both HWDGE rings: SP carries the
            # early chunk-2 planes + chunk-1, Activation the late c2 planes.
            sl2 = slab_pool.tile(list(SLAB2_SHAPE), f32r, tag="slab2",
                                 name="slab_c2")
            sl1 = slab_pool.tile(list(SLAB1_SHAPE), f32r, tag="slab",
                                 name="slab_c1")
            nc.sync.dma_start(sl2[:, 0:2], in2e[:, 0:2])
            w2all = wpool.tile([128, 49 * W2_COLS], f32r, tag="w2a",
                               name="w2all", bufs=1)
            nc.sync.dma_start(w2all[:, :24 * W2_COLS], w2_in[:, :24 * W2_COLS])
            nc.sync.dma_start(w2all[:, 24 * W2_COLS:], w2_in[:, 24 * W2_COLS:])

            # start=True clears the WHOLE psum bank, so open each bank once
            # with a zero-weight full-bank matmul (also a WAW dep that orders
            # it before every accumulate); all real matmuls use start=False.
            # rhs is a memset zeros tile so the opens run before any DMA lands.
            zw_f = stat.tile([128, 128], f32, tag="zw")
            zr_f = stat.tile([128, 512], f32, tag="zr")
            nc.vector.memset(zw_f[:], 0.0)
            nc.vector.memset(zr_f[:], 0.0)
            zw = zw_f.bitcast(f32r)
            zr = zr_f.bitcast(f32r)
            for t in range(8):
                nc.tensor.matmul(pq[t].rearrange("c a y z -> c (a y z)"),
                                 zw[:], zr[:], start=True, stop=False)

            # all chunk-2 weights live in ONE resident tile: a [128 x 25KB]
            # DMA is 128 large descriptors (descriptor-efficient), so chunk-2
            # never starves; chunk-1 tiles trickle on the slow Activation
            # ring, which keeps bulk SBUF writes from contending with the
            # fast-mode matmul stream
            for x in range(2, 10, 2):
                nc.sync.dma_start(sl2[:, x:x + 2], in2e[:, x:x + 2])
            nc.sync.dma_start(sl1[:, 0:11], in1[:, 0:11])
            nc.sync.dma_start(sl1[:, 11:NXS], in1[:, 11:NXS])
            wat = {}
            for i in range(49):
                w = wpool.tile([128, WA_COLS], f32r, tag="wa", name=f"wa_{i}",
                               bufs=4)
                nc.scalar.dma_start(w[:], wa_in[i])
                wat[i] = w


            for n, (src, i, wc, ww, x0, bank, npos) in enumerate(_PLAN):
                ky, kz = i // 7, i % 7
                bx = _box(ky, kz)
                if src == 'c2':
                    lhsT = w2all[:, i * W2_COLS + wc:i * W2_COLS + wc + ww]
                    sl, step = sl2, 1
                else:
                    lhsT = wat[i][:, wc:wc + ww]
                    sl, step = sl1, 2
                yslice = slice(bx['iy0'], bx['iy0'] + 2 * bx['oyc'] - 1, 2)
                if npos == 2:
                    rhs = sl[:, x0:x0 + step + 1:step, yslice, bx['p'],
                             bx['zs']:bx['zs'] + 16]
                    out_ap = pq[bank][0:ww, 0:2,
                                      bx['oy0']:bx['oy0'] + bx['oyc'], :]
                else:
                    rhs = sl[:, x0, yslice, bx['p'], bx['zs']:bx['zs'] + 16]
                    out_ap = pq[bank][0:ww, 0,
                                      bx['oy0']:bx['oy0'] + bx['oyc'], :]
                nc.tensor.matmul(out_ap, lhsT, rhs,
                                 start=False, stop=n in _STOPS)

            # evacuate: plane j = L(j) + U(j+1) [+ LB(10+j-2) j>=2]
            #                                   [+ UB(10+j)   j<=5]
            # a TensorTensor may read only ONE psum operand: Scalar engine
            # copies psum->sbuf, Vector accumulates the second psum operand
            osb = outp.tile([CO, OXC, 16, 16], f32, tag="osb")
            usb = outp.tile([128, OXC, 16, 16], f32, tag="usb")
            for j in range(OXC):
                nc.scalar.activation(usb[0:64, j], pq[j // 2][0:64, j % 2],
                                     mybir.ActivationFunctionType.Copy,
                                     scale=1.0)
                if j >= 2:
                    s = 8 + j
                    nc.vector.tensor_add(usb[0:64, j], usb[0:64, j],
                                         pq[s // 2][0:64, s % 2])
                su = j + 1
                nc.scalar.activation(usb[64:128, j],
                                     pq[su // 2][64:128, su % 2],
                                     mybir.ActivationFunctionType.Copy,
                                     scale=1.0)
                if j <= 5:
                    s = 10 + j
                    nc.vector.tensor_add(usb[64:128, j], usb[64:128, j],
                                         pq[s // 2][64:128, s % 2])
            # rendezvous: a 4-byte token dependent on the first evacuated
            # plane enters a dummy AllReduce, aligning the cores while the
            # evac/stats chain runs, so the real AllReduce's peer wait
            # overlaps work instead of extending the tail
            cc_b_in = dram.tile([1, 64], f32, tag="ccbi")
            cc_b_out = dram.tile([1, 64], f32, tag="ccbo")
            nc.sync.dma_start(cc_b_in[0:1, 0:1], usb[0:1, 0, 0, 0:1])
            nc.gpsimd.collective_compute(
                "AllReduce", mybir.AluOpType.add,
                replica_groups=[list(range(NCORES))],
                ins=[cc_b_in.opt()], outs=[cc_b_out.opt()],
            )

            # fold L+U across partitions on the PE array: psum banks are free
            # after evacuation, and out[c] = usb[c] + usb[64+c] is one fp32
            # matmul with [I;I] stationary per 512-column chunk
            uf = usb.rearrange("c x y z -> c (x y z)")
            for q in range(4):
                nc.tensor.matmul(pq[q][0:64, 0:2, :, :], fold_t[:],
                                 uf[:, 512 * q:512 * (q + 1)],
                                 start=True, stop=True)

            # per-channel sum of squares straight from psum (Square with
            # free-axis accumulator); AllReduce carries variance directly
            sq = outp.tile([CO, 2048], f32, tag="sq")
            ssq = stat.tile([CO, 1], f32, tag="ssq")
            ssq_p = [stat.tile([CO, 1], f32, tag=f"ssqp{q}", name=f"ssqp{q}")
                     for q in range(4)]
            for q in range(4):
                nc.scalar.activation(sq[:, 512 * q:512 * (q + 1)],
                                     pq[q][0:64, 0:2, :, :],
                                     mybir.ActivationFunctionType.Square,
                                     scale=1.0, accum_out=ssq_p[q][:, :])
            nc.vector.tensor_add(ssq[:], ssq_p[0][:], ssq_p[1][:])
            nc.vector.tensor_add(ssq_p[2][:], ssq_p[2][:], ssq_p[3][:])
            nc.vector.tensor_add(ssq[:], ssq[:], ssq_p[2][:])
            ss_row = stat.tile([1, 64], f32, tag="ssrow")
            vloc = stat.tile([1, 64], f32, tag="vloc")
            tmp16 = stat.tile([1, 16], f32, tag="tmp16")
            ss_dram = dram.tile([1, 64], f32, tag="ssd")
            nc.sync.dma_start(ss_dram[0, :], ssq[:, 0])
            nc.sync.dma_start(ss_row[:], ss_dram[:])
            nc.vector.tensor_add(tmp16[:], ss_row[:, 16::3], ss_row[:, 17::3])
            nc.vector.tensor_add(tmp16[:], tmp16[:], ss_row[:, 18::3])
            nc.vector.tensor_scalar_mul(vloc[:, 0:16], ss_row[:, 0:16], VAR_S_DIV)
            for j in range(3):
                nc.vector.tensor_scalar_mul(vloc[:, 16 + j::3], tmp16[:], VAR_V_DIV)

            v_dram = dram.tile([1, 64], f32, tag="vd")
            v_red = dram.tile([1, 64], f32, tag="vr")
            nc.sync.dma_start(v_dram[:], vloc[:])
            nc.gpsimd.collective_compute(
                "AllReduce", mybir.AluOpType.add,
                replica_groups=[list(range(NCORES))],
                ins=[v_dram.opt()], outs=[v_red.opt()],
            )

            # scale = gamma / sqrt(var + eps), in per-partition layout
            var_col = stat.tile([CO, 1], f32, tag="varcol")
            nc.sync.dma_start(var_col[:, 0], v_red[0, :])
            eps_t = stat.tile([CO, 1], f32, tag="eps")
            nc.vector.memset(eps_t[:], EPS)
            sd = stat.tile([CO, 1], f32, tag="sd")
            nc.scalar.activation(sd[:], var_col[:], mybir.ActivationFunctionType.Sqrt,
                                 bias=eps_t[:], scale=1.0)
            inv = stat.tile([CO, 1], f32, tag="inv")
            nc.vector.reciprocal(inv[:], sd[:])
            scale_col = stat.tile([CO, 1], f32, tag="sccol")
            nc.vector.tensor_mul(scale_col[:], inv[:], gam_t[:])

            # apply BN scale while evacuating the psum fold result to osb,
            # then bias+relu on scalar channels; ship each half when ready
            of = osb.rearrange("c x y z -> c (x y z)")
            for q in range(4):
                nc.vector.tensor_scalar_mul(of[:, 512 * q:512 * (q + 1)],
                                            pq[q][0:64, 0:2, :, :],
                                            scale_col[:, :])
            nc.scalar.activation(of[0:16, :], of[0:16, :],
                                 mybir.ActivationFunctionType.Relu,
                                 bias=bias_t[0:16, :], scale=1.0)
            nc.sync.dma_start(out_d[16:64], osb[16:64])
            nc.sync.dma_start(out_d[0:16], osb[0:16])

    nc.compile()
    return nc


def _install_ntff_hook():
    import sys, types
    if "antenv.axon_hooks" in sys.modules:
        return
    mod = types.ModuleType("antenv.axon_hooks")
    mod._hook = None
    mod.set_axon_ntff_profile_hook = lambda h: setattr(mod, "_hook", h)
    mod.get_axon_ntff_profile_hook = lambda: mod._hook
    sys.modules["antenv.axon_hooks"] = mod
    try:
        import antenv
        antenv.axon_hooks = mod
        from trn_agent_boot.trn_boot import _ntff_profile_via_ctypes
        mod.set_axon_ntff_profile_hook(_ntff_profile_via_ctypes("/opt/axon/libaxon_pjrt.so"))
    except Exception:
        pass


def run_on_hw(inp, trace=False):
    """Run the kernel on 8 cores. Returns (full output [4,64,16,16,16], results)."""
    from concourse.bass_utils import run_bass_kernel_spmd

    if "nc" not in _CACHED:
        _install_ntff_hook()
        _CACHED["nc"] = _build_bass()
    nc = _CACHED["nc"]

    svt = _svt_sym(inp['sv'])
    Ks = _assemble_kernel_sym(inp)
    WA, W2 = _weight_slabs(Ks)
    gam, bias = _gam_bias(inp['bn_g_s'], inp['bn_g_v'], inp['bias_s'])

    in_maps = []
    for c in range(NCORES):
        b, h = c // 2, c % 2
        c1, c2e = _core_slabs(svt, b, h)
        F = np.zeros((128, 64), np.float32)
        F[np.arange(64), np.arange(64)] = 1.0
        F[64 + np.arange(64), np.arange(64)] = 1.0
        in_maps.append({
            "in1": c1,
            "in2e": c2e,
            "fold_in": F,
            "wa_in": WA,
            "w2_in": np.ascontiguousarray(W2.transpose(1, 0, 2).reshape(128, 49 * W2_COLS)),
            "gam_in": gam.reshape(64, 1),
            "bias_in": bias.reshape(64, 1),
        })

    res = run_bass_kernel_spmd(nc, in_maps, core_ids=list(range(NCORES)), trace=trace)

    y = np.zeros((B, CO, 16, 16, 16), np.float32)
    for c in range(NCORES):
        b, h = c // 2, c % 2
        y[b, :, 8 * h:8 * h + 8] = res.results[c]["out"]
    return y, res


def kernel(**inputs) -> np.ndarray:
    y, _ = run_on_hw(inputs, trace=False)
    return y
